# revision 37
# baseline (speedup 1.0000x reference)
"""Trainium2 Bass kernel for nn_Attention (general-score attention with
masked softmax), data-parallel over batch across 8 NeuronCores.

Math (per batch), matching the reference exactly for {0,1} float masks:
    raw[t,s]  = sum_e (hidden @ W)[t,e] * enc[s,e]       (associativity trick:
                (hidden @ W) @ enc^T  ==  hidden @ (enc @ W^T)^T, saves 25%
                FLOPs and avoids materializing proj)
    attn_energies = raw * mask            (mask in {0,1} so mask^2 == mask)
    e = exp(x - max_s x) * mask
    attn = e / (sum_s e + 1e-6)
    context = attn @ enc_value

v10 (95.3us), built up from ntff analysis of v3 (105.9us) through v9.
Measured facts this version is built on: 512-wide fp16 matmuls issue
back-to-back at 216ns warm (LDWEIGHTS hides); Sync HW-DGE dma_start
issue costs ~0.68us serial; DMA completion-to-semaphore lag ~0.8us;
GpSimd SW-DGE issues cost ~5us (never use); a parallel Scalar-queue
prefetch starves the critical stream at the HBM; the graded window runs
from the first module instruction to the last framework-epilogue
instruction (the ~7.7us full-semaphore-file zeroing is a fixed tax);
fp8 e4m3 measures 3.7% matmul rel err (budget 2e-2) so fp16 is the
fastest legal dtype; occasional runs land in P0 power state (PE at
2.0GHz, 259ns/MM) — rerun after a cooldown to compare fairly.

Changes vs v3:
  - s-compaction: the {0,1} source mask keeps ~86% of the 1024 source
    positions.  The host gathers the valid s-columns per batch (encT, val)
    into a compact SC-wide layout (SC = max valid count over batches,
    rounded up to 128; 896 for the graded inputs), the kernel computes
    energies/softmax/ctx over SC columns only, and the host scatters the
    ae/aw outputs back to full width with exact zeros at masked positions
    (the reference's masked entries are exact zeros).  Saves 12.5% of the
    mm2 streaming cycles and 1/8 of the mm3 matmuls + transposes.  The row
    max over the compact tile equals the reference's max over x*mask
    exactly (padded columns give raw energy 0 = the reference's masked 0).
  - (w-pass1-half, hidT) pairs fused into single 0.375MB DMAs via a
    host-interleaved whidA[(dt, p, 3, 512)] layout; pass-2's w halves
    ride as ONE fused 1MB tensor.  mm1's supply-critical stream is 25%
    lighter and needs 10 Sync issues instead of 16.  All loads stay on
    Sync in consumption order (supply order whidA0a, whidA1, whidA0b
    matches the reordered round-0-h0 / round-1 / round-0-h1 consumption;
    f32 psum accumulation is order-independent).
  - entry-barrier hoist: the three supply-critical DMA issues (and the
    junk-scratch memset, on DVE) are relocated between their engine's
    barrier-arrival DRAIN and barrier-release wait, so the first DMA
    issues at ~6.6us instead of ~7.2us and mm1 starts at ~9.7us.
  - junk HAM-warmup transposes sized (28) to end right as the first pair's
    data+semaphore land; an idle gap before mm1 is doubly bad (the wait
    itself plus ~12 cold 427ns matmuls, HAM's warm transition being
    absolute-time).  The Scalar HW-DGE queue measured a ~4us first-use
    cold-start, so supply-critical loads must stay on Sync's (preamble-
    warmed) queue.
  - last ctx tile drains in shrinking chunks (384/256/256/128) in four
    SEPARATE single-bank psum ring tiles — chunks sharing a tile
    serialize on the tile framework's write-after-read tracking (v5
    measured 1.9us of stalls); the end-of-kernel tail is one 128-wide
    DVE copy + tiny DMA.  ctx_sb ring deepened to 3.
  - everything else as v3: fp16 gemms with f32 PSUM accumulation, fused
    two-batch mm1 with dt-outer DMA-paired consumption and staggered
    et-wise drains, PE-transposes of attn two tiles ahead of mm3, packed
    [ae|aw] output tiles.
"""
import os

import numpy as np

B, TRG, SRC, ENCD, TRGD = 16, 512, 1024, 1024, 1024
NCORES = 8
BPC = B // NCORES  # batches per core
P = 128
nD = TRGD // P   # 8 contraction tiles over d
nE = ENCD // P   # 8 over e
nT = TRG // P    # 4 t-tiles per batch
TRG2 = BPC * TRG  # both batches fused along t: 1024

_cache = {}

LAST_EXEC_NS = None
LAST_RESULTS = None


def _build(SC):
    import bass_rust
    import concourse.mybir as mybir
    import concourse.tile as tile
    from concourse import bacc
    from concourse.masks import make_identity

    _add_dep = bass_rust.add_dep_helper

    F32 = mybir.dt.float32
    FP16 = mybir.dt.float16
    ALU = mybir.AluOpType
    AXL = mybir.AxisListType
    ACT_EXP = mybir.ActivationFunctionType.Exp

    nSC = SC // P  # compacted source tiles
    # mm2 moving-operand chunks over the SC free dim (PSUM banks are 512
    # f32, so chunk boundaries stay bank-aligned at 512)
    sc_chunks = []
    off = 0
    while off < SC:
        w = min(512, SC - off)
        sc_chunks.append((off, off + w))
        off += w

    nc = bacc.Bacc("TRN2", target_bir_lowering=False, debug=False)

    whidA_d = nc.dram_tensor("whidA", (nD, P, 3, 512), FP16,
                             kind="ExternalInput")
    wB_d = nc.dram_tensor("wB", (P, nD, 512), FP16, kind="ExternalInput")
    encT_d = nc.dram_tensor("encT", (BPC, P, nE, SC), FP16,
                            kind="ExternalInput")
    val_d = nc.dram_tensor("val", (BPC, P, nSC, TRGD), FP16,
                           kind="ExternalInput")
    mask_d = nc.dram_tensor("mask", (BPC, 1, SC), FP16, kind="ExternalInput")
    aeaw_d = nc.dram_tensor("aeaw", (BPC, TRG, 2 * SC), FP16,
                            kind="ExternalOutput")
    ctx_d = nc.dram_tensor("ctx", (BPC, TRG, TRGD), FP16,
                           kind="ExternalOutput")

    with tile.TileContext(nc) as tc:
        with (
            tc.tile_pool(name="const", bufs=1) as const,
            tc.tile_pool(name="big", bufs=1) as big,
            tc.tile_pool(name="sm", bufs=2) as sm,
            tc.tile_pool(name="ps", bufs=4, space="PSUM") as psp,
        ):
            # junk-warmup scratch: a bare memset is ready ~1.3us before the
            # identity (memset+affine_select+copy), so the HAM warmup can
            # start that much sooner.  On DVE so it can be hoisted before
            # the entry barrier (see the block surgery below).
            scr = const.tile([P, P], FP16)
            scr_set = nc.vector.memset(scr[:], 0.0)
            ident = const.tile([P, P], F32)
            make_identity(nc, ident[:])
            identh = const.tile([P, P], FP16)
            nc.vector.tensor_copy(identh[:], ident[:])

            # PE program order is pinned with an explicit linear chain so the
            # scheduler can never interleave accumulation groups or delay a
            # group's stop.
            pe_prev = [None]

            def chain(mm):
                if pe_prev[0] is not None:
                    _add_dep(mm.ins, pe_prev[0].ins, sync=False,
                             reason="pe order")
                pe_prev[0] = mm
                return mm

            # ---- loads (issue order == consumption order) ----
            # whidA[dt] carries w[dt] cols 0:512 (all pass-1 needs) plus the
            # full hidT[dt]; pass-2's w halves ride later as ONE fused 1MB
            # DMA.  This thins mm1's supply-critical stream by 25% and cuts
            # Sync's serial issue count (measured ~0.68us per dma_start,
            # ~0.8us DMA-completion-to-semaphore lag).
            whidA_sb = [big.tile([P, 3, 512], FP16, tag=f"whidA{i}",
                                 name=f"whidA_sb{i}") for i in range(nD)]
            wB_sb = big.tile([P, nD, 512], FP16, tag="wB", name="wB_sb")
            # whidA[0] split: (w0-half + hid0-h0) first so round0-h0 starts
            # on 0.25MB; hid0-h1 follows whidA[1] in supply order to match
            # the reordered round-0/round-1 consumption below
            # All loads on Sync's HW-DGE in consumption order.  The Scalar
            # queue (Q10) measured a ~4us first-use cold-start lag, so it
            # must NOT carry supply-critical loads; Sync's queue is warmed
            # by the framework preamble.  The first three issues are
            # hoisted before the entry barrier's release-wait (Sync's
            # barrier-arrival drain completes ~6.2us, ~0.4us before the
            # release fires).
            early_dmas = [
                nc.sync.dma_start(out=whidA_sb[0][:, 0:2, :],
                                  in_=whidA_d[0][:, 0:2, :]),
                nc.sync.dma_start(out=whidA_sb[1][:], in_=whidA_d[1]),
                nc.sync.dma_start(out=whidA_sb[0][:, 2, :],
                                  in_=whidA_d[0][:, 2, :]),
            ]
            for i in range(2, nD):
                nc.sync.dma_start(out=whidA_sb[i][:], in_=whidA_d[i])
            nc.sync.dma_start(out=wB_sb[:], in_=wB_d[:])
            maskbs = []
            for b in range(BPC):
                maskb_hf = sm.tile([P, SC], FP16, tag="maskb_hf",
                                   name=f"maskb_hf{b}")
                nc.sync.dma_start(out=maskb_hf[:],
                                  in_=mask_d[b].to_broadcast((P, SC)))
                maskbs.append(maskb_hf)
            # encT/val are host-compacted to valid s-columns (padding zeros)
            # and marshaled partition-major, one DMA each per batch
            encT_sb = []
            val_sb = []
            for b in range(BPC):
                e_t = big.tile([P, nE, SC], FP16, tag="encT", bufs=2,
                               name=f"encT_sb{b}")
                nc.sync.dma_start(out=e_t[:], in_=encT_d[b])
                v_t = big.tile([P, nSC, TRGD], FP16, tag="val", bufs=2,
                               name=f"val_sb{b}")
                nc.sync.dma_start(out=v_t[:], in_=val_d[b])
                encT_sb.append(e_t)
                val_sb.append(v_t)

            # ---- mm1: HpT[e, t01] = sum_d W[d,e] * hidT01[d, t01] ----
            # two half-passes of 4 et each (4 psum bufs per pass).  dt-outer
            # for DMA pair-wise consumption, but the last two dt rounds go
            # et-wise with the drain right after each stop so ring slots free
            # up staggered instead of all at the end.
            HpT = big.tile([P, nE, TRG2], FP16, tag="HpT", name="HpT")
            drain_eng = [0]

            def drain(dst, src):
                if drain_eng[0] % 2 == 0:
                    nc.vector.tensor_copy(dst, src)
                else:
                    nc.scalar.copy(dst, src)
                drain_eng[0] += 1

            def w_ap(dt, et):
                if et < 4:
                    return whidA_sb[dt][:, 0, et * P:(et + 1) * P]
                return wB_sb[:, dt, (et - 4) * P:(et - 3) * P]

            def mm1_mm(pp, dt, et):
                for h in range(2):
                    hs = slice(h * 512, (h + 1) * 512)
                    chain(nc.tensor.matmul(
                        pp[:, hs], w_ap(dt, et),
                        whidA_sb[dt][:, 1 + h, :],
                        start=(dt == 0), stop=(dt == nD - 1)))

            def emit_mm1_pass(ets, warm=False):
                pps = [psp.tile([P, TRG2], F32, tag="ps", name=f"mm1ps{et}")
                       for et in ets]
                if warm:
                    # junk transposes of the identity: keep the PE busy
                    # during the DMA/preamble dead time so the HAM clock
                    # gate is warm (2.4GHz) when real work arrives, sized
                    # to end right as whid[0]'s first half lands (~10.2us).
                    # An idle gap here is doubly bad: the wait itself plus
                    # ~11 cold 427ns matmuls after it (v5 measured).  The
                    # garbage psum is overwritten by mm1's start=True.
                    junk_view = pps[0][:].bitcast(FP16)
                    for _ in range(28):
                        chain(nc.tensor.matmul(
                            junk_view[:, 0:P], scr[:], scr[:],
                            is_transpose=True, skip_group_check=True))
                    # supply-ordered head: round0-h0 (whidA[0]'s first
                    # 2/3rds), then ALL of round 1 (whidA[1], issued 2nd),
                    # then round0-h1 (hid0-h1, issued 3rd).  f32 psum
                    # accumulation is order-independent; bank B's group is
                    # opened (start=True) by dt1-h1 since it now runs first.
                    h0, h1 = slice(0, 512), slice(512, 1024)
                    for i, et in enumerate(ets):
                        chain(nc.tensor.matmul(
                            pps[i][:, h0], w_ap(0, et),
                            whidA_sb[0][:, 1, :], start=True, stop=False))
                    for i, et in enumerate(ets):
                        chain(nc.tensor.matmul(
                            pps[i][:, h0], w_ap(1, et),
                            whidA_sb[1][:, 1, :], start=False, stop=False))
                        chain(nc.tensor.matmul(
                            pps[i][:, h1], w_ap(1, et),
                            whidA_sb[1][:, 2, :], start=True, stop=False))
                    for i, et in enumerate(ets):
                        chain(nc.tensor.matmul(
                            pps[i][:, h1], w_ap(0, et),
                            whidA_sb[0][:, 2, :], start=False, stop=False))
                    dt_start = 2
                else:
                    dt_start = 0
                for dt in range(dt_start, nD - 2):
                    for i, et in enumerate(ets):
                        mm1_mm(pps[i], dt, et)
                for i, et in enumerate(ets):
                    for dt in (nD - 2, nD - 1):
                        mm1_mm(pps[i], dt, et)
                    drain(HpT[:, et, :], pps[i][:])

            emit_mm1_pass(range(0, nE // 2), warm=True)
            emit_mm1_pass(range(nE // 2, nE))

            # ---- mm2 + masked softmax over 8 supertiles (b, tt) ----
            tiles = [(b, tt) for b in range(BPC) for tt in range(nT)]
            pks = []
            attnTs = {}

            def emit_mm2(k):
                b, tt = tiles[k]
                ts = slice(b * TRG + tt * P, b * TRG + (tt + 1) * P)
                en_ps = psp.tile([P, SC], F32, tag="ps", name=f"en{b}{tt}")
                for et in range(nE):
                    for c0, c1 in sc_chunks:
                        chain(nc.tensor.matmul(en_ps[:, c0:c1],
                                               HpT[:, et, ts],
                                               encT_sb[b][:, et, c0:c1],
                                               start=(et == 0),
                                               stop=(et == nE - 1)))
                return en_ps

            def emit_softmax(k, en_ps):
                b, tt = tiles[k]
                maskb_hf = maskbs[b]
                # packed [ae | attn] tile: one output DMA per supertile
                pk = sm.tile([P, 2 * SC], FP16, tag="aeaw", bufs=7,
                             name=f"aeaw{b}{tt}")
                nc.scalar.copy(pk[:, :SC], en_ps[:])
                negm = sm.tile([P, 1], F32, tag="negm")
                nc.vector.tensor_reduce(negm[:], en_ps[:], axis=AXL.X,
                                        op=ALU.max, negate=True)
                ex = sm.tile([P, SC], FP16, tag="ex")
                nc.scalar.activation(ex[:], en_ps[:], ACT_EXP, bias=negm[:],
                                     scale=1.0)
                rowsum = sm.tile([P, 1], F32, tag="rowsum")
                nc.vector.scalar_tensor_tensor(ex[:], ex[:], 1.0,
                                               maskb_hf[:],
                                               op0=ALU.mult, op1=ALU.mult,
                                               accum_out=rowsum[:])
                z = sm.tile([P, 1], F32, tag="z")
                nc.vector.tensor_scalar_add(z[:], rowsum[:], 1e-6)
                rz = sm.tile([P, 1], F32, tag="rz")
                nc.vector.reciprocal(rz[:], z[:])
                nc.vector.tensor_scalar_mul(pk[:, SC:], ex[:], rz[:])
                nc.sync.dma_start(out=aeaw_d[b, tt * P:(tt + 1) * P, :],
                                  in_=pk[:])
                pks.append(pk)

            # leading supertiles transposed via DMA xbar; 2 keeps Sync's
            # B-phase issue budget (7 xbar issues each + 8 aeaw) under the
            # phase length so phase-C ctx DMAs never queue late
            N_XBAR = 2

            def emit_tr(k):
                # PE transposes for the trailing supertiles (the xbar path
                # can't make their deadlines); ~56ns each on the PE chain
                attn = pks[k][:, SC:]
                trp = psp.tile([P, SC], F32, tag="ps", name=f"tr{k}")
                trh = trp[:].bitcast(FP16)
                for st in range(nSC):
                    chain(nc.tensor.transpose(trh[:, st * P:(st + 1) * P],
                                              attn[:, st * P:(st + 1) * P],
                                              identh[:]))
                attnT = sm.tile([P, nSC, P], FP16, tag="attnT",
                                name=f"attnT{k}")
                nc.scalar.copy(attnT[:], trh[:, :SC])
                attnTs[k] = attnT

            def emit_tr_xbar(k):
                # DMA-xbar transposes for the LEADING supertiles: ~5-6us
                # per supertile on Sync's queue, but attn(k) for small k is
                # ready ~20us before mm3(k) needs it, so the latency hides
                # completely and the PE chain drops 7 transposes + the
                # Scalar attnT copy per supertile
                attn = pks[k][:, SC:]
                attnT = sm.tile([P, nSC, P], FP16, tag="attnT",
                                name=f"attnT{k}")
                for st in range(nSC):
                    nc.sync.dma_start(out=attnT[:, st, :],
                                      in_=attn[:, st * P:(st + 1) * P],
                                      transpose=True)
                attnTs[k] = attnT

            for k in range(len(tiles)):
                en_ps = emit_mm2(k)
                emit_softmax(k, en_ps)
                if k < N_XBAR:
                    emit_tr_xbar(k)

            def emit_mm3(k):
                b, tt = tiles[k]
                attnT = attnTs.pop(k)
                last = (k == len(tiles) - 1)
                ctx_sb = sm.tile([P, TRGD], FP16, tag="ctx_sb", bufs=3)
                rows = slice(tt * P, (tt + 1) * P)
                if last:
                    # shrinking chunks in 4 SEPARATE single-bank psum ring
                    # tiles (chunks sharing a tile serialize: the tile
                    # framework's write-after-read tracking made chunk c+1's
                    # matmuls wait for chunk c's drain — v5 measured 1.9us of
                    # stalls).  Each chunk drains and ships while the next
                    # multiplies in a different bank/tile, so the end-of-
                    # kernel tail is one 128-wide DVE copy + tiny DMA.
                    edges = [0, 384, 640, 896, 1024]
                    for c in range(4):
                        tp = psp.tile([P, 512], F32, tag="ps",
                                      name=f"ctx7_{c}")
                        wdt = edges[c + 1] - edges[c]
                        ps_sl = slice(0, wdt)
                        out_sl = slice(edges[c], edges[c + 1])
                        for st in range(nSC):
                            chain(nc.tensor.matmul(tp[:, ps_sl],
                                                   attnT[:, st, :],
                                                   val_sb[b][:, st, out_sl],
                                                   start=(st == 0),
                                                   stop=(st == nSC - 1)))
                        nc.vector.tensor_copy(ctx_sb[:, out_sl],
                                              tp[:, ps_sl])
                        if c < 3:
                            nc.sync.dma_start(out=ctx_d[b, rows, out_sl],
                                              in_=ctx_sb[:, out_sl])
                        else:
                            nc.scalar.dma_start(out=ctx_d[b, rows, out_sl],
                                                in_=ctx_sb[:, out_sl])
                    return
                ctx_ps = psp.tile([P, TRGD], F32, tag="ps", name=f"ctx{k}")
                for st in range(nSC):
                    for h in range(2):
                        hs = slice(h * 512, (h + 1) * 512)
                        chain(nc.tensor.matmul(ctx_ps[:, hs],
                                               attnT[:, st, :],
                                               val_sb[b][:, st, hs],
                                               start=(st == 0),
                                               stop=(st == nSC - 1)))
                # DVE, not Scalar: the Scalar FIFO still holds late
                # attn-transpose issues during early mm3 tiles
                nc.vector.tensor_copy(ctx_sb[:], ctx_ps[:])
                nc.sync.dma_start(out=ctx_d[b, rows, :], in_=ctx_sb[:])

            # PE transposes run two tiles ahead of mm3: each attnT Scalar
            # copy then has two full mm3 rounds of cover before its consumer
            for k in range(len(tiles)):
                if N_XBAR <= k + 2 < len(tiles):
                    emit_tr(k + 2)
                emit_mm3(k)

    # ---- entry-barrier hoist ----
    # The Bacc-init all-engine barrier gates the first DMA issue at ~7.2us
    # into the graded window while Sync sits idle from ~5.9us.  Hoist the
    # three supply-critical DMA issues to between Sync's barrier-arrival
    # DRAIN and its barrier-release wait (after the drain, so the barrier
    # never waits on the transfers), and the junk-scratch memset likewise on
    # DVE.  Semaphores are zeroed by the NEFF preamble before any module
    # instruction, and each hoisted instruction carries its sync_info with
    # it; instructions with semaphore waits are left in place.
    il0 = nc.main_func.blocks[0].instructions

    def hoist_before(inst, anchor_name_prefix):
        # Move a body-block instruction into the init block, right before
        # its engine's entry-barrier release-wait (and hence after that
        # engine's barrier-arrival DRAIN — a DRAIN waits on the engine's
        # outstanding DMAs, so issuing before it would stall the barrier).
        dbg = os.environ.get("KERNEL_HOIST_DEBUG")
        si = inst.sync_info
        if si is not None and len(si.on_wait) > 0:
            if dbg:
                print(f"HOIST-SKIP waits {inst.name}: {si.on_wait}")
            return
        src = None
        for b in nc.main_func.blocks:
            if inst in b.instructions:
                src = b
                break
        if src is None:
            if dbg:
                print(f"HOIST-SKIP notfound {inst.name}")
            return
        for k, x in enumerate(il0):
            if type(x).__name__ == "InstEventSemaphore" and str(
                    x.name).startswith(anchor_name_prefix):
                src.instructions.remove(inst)
                il0.insert(k, inst)
                if dbg:
                    print(f"HOIST-OK {inst.name} -> before {x.name} @{k}")
                return
        if dbg:
            print(f"HOIST-SKIP noanchor {inst.name}")

    hoist_before(scr_set.ins, "barrier_DVE_")
    for d in early_dmas:
        hoist_before(d.ins, "barrier_SP_")

    nc.compile()
    return nc


def kernel(hidden, encoder_outputs, encoder_value, encoder_mask, W):
    global LAST_EXEC_NS, LAST_RESULTS
    from concourse.bass_utils import run_bass_kernel_spmd

    hidden = np.ascontiguousarray(hidden, dtype=np.float32)
    encoder_outputs = np.ascontiguousarray(encoder_outputs, dtype=np.float32)
    encoder_value = np.ascontiguousarray(encoder_value, dtype=np.float32)
    encoder_mask = np.ascontiguousarray(encoder_mask, dtype=np.float32)
    W = np.ascontiguousarray(W, dtype=np.float32)

    # s-compaction: gather valid source positions per batch; SC is the max
    # valid count rounded up to 128 (uniform across cores — SPMD shares one
    # compiled program)
    idx = [np.nonzero(encoder_mask[b] > 0.5)[0] for b in range(B)]
    n_max = max(len(ix) for ix in idx)
    SC = min(SRC, -(-max(n_max, 1) // P) * P)
    nSC = SC // P

    if ("nc", SC) not in _cache:
        _cache[("nc", SC)] = _build(SC)
    nc = _cache[("nc", SC)]

    w_tiles = W.astype(np.float16).reshape(nD, P, ENCD)
    # (P, nD, 512) to match the SBUF tile's linear order — DMA does not
    # transpose
    wB = np.ascontiguousarray(w_tiles[:, :, 512:].transpose(1, 0, 2))
    in_maps = []
    for c in range(NCORES):
        sl = slice(c * BPC, (c + 1) * BPC)
        hid2 = hidden[sl]  # (2, TRG, TRGD)
        hidT01 = np.concatenate([hid2[0].T, hid2[1].T], axis=1)
        hid_tiles = hidT01.astype(np.float16).reshape(nD, P, TRG2)
        whidA = np.empty((nD, P, 3, 512), dtype=np.float16)
        whidA[:, :, 0, :] = w_tiles[:, :, 0:512]
        whidA[:, :, 1, :] = hid_tiles[:, :, 0:512]
        whidA[:, :, 2, :] = hid_tiles[:, :, 512:]
        encT_pm = np.zeros((BPC, P, nE, SC), dtype=np.float16)
        val_pm = np.zeros((BPC, P, nSC, TRGD), dtype=np.float16)
        maskc = np.zeros((BPC, 1, SC), dtype=np.float16)
        for j in range(BPC):
            b = c * BPC + j
            ix = idx[b]
            n_b = len(ix)
            # encT compact: [ENCD, n_b] -> pad to SC -> partition-major
            encTc = encoder_outputs[b].T[:, ix].astype(np.float16)
            encT_pm[j, :, :, :n_b] = encTc.reshape(nE, P, n_b).transpose(
                1, 0, 2)
            # val compact: [n_b, TRGD] -> pad to SC rows -> partition-major
            valc = np.zeros((SC, TRGD), dtype=np.float16)
            valc[:n_b] = encoder_value[b][ix]
            val_pm[j] = valc.reshape(nSC, P, TRGD).transpose(1, 0, 2)
            maskc[j, 0, :n_b] = 1.0
        in_maps.append({
            "whidA": whidA,
            "wB": wB,
            "encT": encT_pm,
            "val": val_pm,
            "mask": maskc,
        })

    trace = bool(int(os.environ.get("KERNEL_TRACE", "0")))
    res = run_bass_kernel_spmd(nc, in_maps, core_ids=list(range(NCORES)),
                               trace=trace)
    LAST_EXEC_NS = res.exec_time_ns
    LAST_RESULTS = res

    context = np.concatenate([res.results[c]["ctx"] for c in range(NCORES)],
                             axis=0).astype(np.float32)
    # scatter compacted ae/aw back to full source width; masked positions
    # are exact zeros in the reference (energies*mask and e*mask)
    attn_energies = np.zeros((B, TRG, SRC), dtype=np.float32)
    attn_weights = np.zeros((B, TRG, SRC), dtype=np.float32)
    for c in range(NCORES):
        aeaw = res.results[c]["aeaw"]
        for j in range(BPC):
            b = c * BPC + j
            ix = idx[b]
            n_b = len(ix)
            attn_energies[b][:, ix] = aeaw[j][:, :n_b].astype(np.float32)
            attn_weights[b][:, ix] = aeaw[j][:, SC:SC + n_b].astype(
                np.float32)
    return context, attn_weights, attn_energies


# revision 41
# speedup vs baseline: 1.0640x; 1.0640x over previous
"""Trainium2 Bass kernel for nn_Attention (general-score attention with
masked softmax), data-parallel over batch across 8 NeuronCores.

Math (per batch), matching the reference exactly for {0,1} float masks:
    raw[t,s]  = sum_e (hidden @ W)[t,e] * enc[s,e]       (associativity trick:
                (hidden @ W) @ enc^T  ==  hidden @ (enc @ W^T)^T, saves 25%
                FLOPs and avoids materializing proj)
    attn_energies = raw * mask            (mask in {0,1} so mask^2 == mask)
    e = exp(x - max_s x) * mask
    attn = e / (sum_s e + 1e-6)
    context = attn @ enc_value

v10 (95.3us), built up from ntff analysis of v3 (105.9us) through v9.
Measured facts this version is built on: 512-wide fp16 matmuls issue
back-to-back at 216ns warm (LDWEIGHTS hides); Sync HW-DGE dma_start
issue costs ~0.68us serial; DMA completion-to-semaphore lag ~0.8us;
GpSimd SW-DGE issues cost ~5us (never use); a parallel Scalar-queue
prefetch starves the critical stream at the HBM; the graded window runs
from the first module instruction to the last framework-epilogue
instruction (the ~7.7us full-semaphore-file zeroing is a fixed tax);
fp8 e4m3 measures 3.7% matmul rel err (budget 2e-2) so fp16 is the
fastest legal dtype; occasional runs land in P0 power state (PE at
2.0GHz, 259ns/MM) — rerun after a cooldown to compare fairly.

Changes vs v3:
  - s-compaction: the {0,1} source mask keeps ~86% of the 1024 source
    positions.  The host gathers the valid s-columns per batch (encT, val)
    into a compact SC-wide layout (SC = max valid count over batches,
    rounded up to 128; 896 for the graded inputs), the kernel computes
    energies/softmax/ctx over SC columns only, and the host scatters the
    ae/aw outputs back to full width with exact zeros at masked positions
    (the reference's masked entries are exact zeros).  Saves 12.5% of the
    mm2 streaming cycles and 1/8 of the mm3 matmuls + transposes.  The row
    max over the compact tile equals the reference's max over x*mask
    exactly (padded columns give raw energy 0 = the reference's masked 0).
  - (w-pass1-half, hidT) pairs fused into single 0.375MB DMAs via a
    host-interleaved whidA[(dt, p, 3, 512)] layout; pass-2's w halves
    ride as ONE fused 1MB tensor.  mm1's supply-critical stream is 25%
    lighter and needs 10 Sync issues instead of 16.  All loads stay on
    Sync in consumption order (supply order whidA0a, whidA1, whidA0b
    matches the reordered round-0-h0 / round-1 / round-0-h1 consumption;
    f32 psum accumulation is order-independent).
  - entry-barrier hoist: the three supply-critical DMA issues (and the
    junk-scratch memset, on DVE) are relocated between their engine's
    barrier-arrival DRAIN and barrier-release wait, so the first DMA
    issues at ~6.6us instead of ~7.2us and mm1 starts at ~9.7us.
  - junk HAM-warmup transposes sized (28) to end right as the first pair's
    data+semaphore land; an idle gap before mm1 is doubly bad (the wait
    itself plus ~12 cold 427ns matmuls, HAM's warm transition being
    absolute-time).  The Scalar HW-DGE queue measured a ~4us first-use
    cold-start, so supply-critical loads must stay on Sync's (preamble-
    warmed) queue.
  - last ctx tile drains in shrinking chunks (384/256/256/128) in four
    SEPARATE single-bank psum ring tiles — chunks sharing a tile
    serialize on the tile framework's write-after-read tracking (v5
    measured 1.9us of stalls); the end-of-kernel tail is one 128-wide
    DVE copy + tiny DMA.  ctx_sb ring deepened to 3.
  - everything else as v3: fp16 gemms with f32 PSUM accumulation, fused
    two-batch mm1 with dt-outer DMA-paired consumption and staggered
    et-wise drains, PE-transposes of attn two tiles ahead of mm3, packed
    [ae|aw] output tiles.
"""
import os

import numpy as np

B, TRG, SRC, ENCD, TRGD = 16, 512, 1024, 1024, 1024
NCORES = 8
BPC = B // NCORES  # batches per core
P = 128
nD = TRGD // P   # 8 contraction tiles over d
nE = ENCD // P   # 8 over e
nT = TRG // P    # 4 t-tiles per batch
TRG2 = BPC * TRG  # both batches fused along t: 1024

_cache = {}

LAST_EXEC_NS = None
LAST_RESULTS = None


def _build(SC):
    import bass_rust
    import concourse.mybir as mybir
    import concourse.tile as tile
    from concourse import bacc
    from concourse.masks import make_identity

    _add_dep = bass_rust.add_dep_helper

    F32 = mybir.dt.float32
    FP16 = mybir.dt.float16
    ALU = mybir.AluOpType
    AXL = mybir.AxisListType
    ACT_EXP = mybir.ActivationFunctionType.Exp

    nSC = SC // P  # compacted source tiles
    # mm2 moving-operand chunks over the SC free dim (PSUM banks are 512
    # f32, so chunk boundaries stay bank-aligned at 512)
    sc_chunks = []
    off = 0
    while off < SC:
        w = min(512, SC - off)
        sc_chunks.append((off, off + w))
        off += w

    nc = bacc.Bacc("TRN2", target_bir_lowering=False, debug=False)

    whidA_d = nc.dram_tensor("whidA", (nD, P, 3, 512), FP16,
                             kind="ExternalInput")
    wB_d = nc.dram_tensor("wB", (P, nD, 512), FP16, kind="ExternalInput")
    encT_d = nc.dram_tensor("encT", (BPC, P, nE, SC), FP16,
                            kind="ExternalInput")
    val_d = nc.dram_tensor("val", (BPC, P, nSC, TRGD), FP16,
                           kind="ExternalInput")
    mask_d = nc.dram_tensor("mask", (BPC, 1, SC), FP16, kind="ExternalInput")
    aeaw_d = nc.dram_tensor("aeaw", (BPC, TRG, 2 * SC), FP16,
                            kind="ExternalOutput")
    ctx_d = nc.dram_tensor("ctx", (BPC, TRG, TRGD), FP16,
                           kind="ExternalOutput")

    with tile.TileContext(nc) as tc:
        with (
            tc.tile_pool(name="const", bufs=1) as const,
            tc.tile_pool(name="big", bufs=1) as big,
            tc.tile_pool(name="sm", bufs=2) as sm,
            tc.tile_pool(name="ps", bufs=4, space="PSUM") as psp,
        ):
            # junk-warmup scratch: a bare memset is ready ~1.3us before the
            # identity (memset+affine_select+copy), so the HAM warmup can
            # start that much sooner.  On DVE so it can be hoisted before
            # the entry barrier (see the block surgery below).
            scr = const.tile([P, P], FP16)
            scr_set = nc.vector.memset(scr[:], 0.0)
            ident = const.tile([P, P], F32)
            make_identity(nc, ident[:])
            identh = const.tile([P, P], FP16)
            nc.vector.tensor_copy(identh[:], ident[:])

            # PE program order is pinned with an explicit linear chain so the
            # scheduler can never interleave accumulation groups or delay a
            # group's stop.
            pe_prev = [None]
            junk_mms = []

            def chain(mm):
                if pe_prev[0] is not None:
                    _add_dep(mm.ins, pe_prev[0].ins, sync=False,
                             reason="pe order")
                pe_prev[0] = mm
                return mm

            # ---- loads (issue order == consumption order) ----
            # whidA[dt] carries w[dt] cols 0:512 (all pass-1 needs) plus the
            # full hidT[dt]; pass-2's w halves ride later as ONE fused 1MB
            # DMA.  This thins mm1's supply-critical stream by 25% and cuts
            # Sync's serial issue count (measured ~0.68us per dma_start,
            # ~0.8us DMA-completion-to-semaphore lag).
            whidA_sb = [big.tile([P, 3, 512], FP16, tag=f"whidA{i}",
                                 name=f"whidA_sb{i}") for i in range(nD)]
            wB_sb = big.tile([P, nD, 512], FP16, tag="wB", name="wB_sb")
            # whidA[0] split: (w0-half + hid0-h0) first so round0-h0 starts
            # on 0.25MB; hid0-h1 follows whidA[1] in supply order to match
            # the reordered round-0/round-1 consumption below
            # All loads on Sync's HW-DGE in consumption order.  The Scalar
            # queue (Q10) measured a ~4us first-use cold-start lag, so it
            # must NOT carry supply-critical loads; Sync's queue is warmed
            # by the framework preamble.  The first three issues are
            # hoisted before the entry barrier's release-wait (Sync's
            # barrier-arrival drain completes ~6.2us, ~0.4us before the
            # release fires).
            early_dmas = [
                nc.sync.dma_start(out=whidA_sb[0][:, 0:2, :],
                                  in_=whidA_d[0][:, 0:2, :]),
                nc.sync.dma_start(out=whidA_sb[1][:], in_=whidA_d[1]),
                nc.sync.dma_start(out=whidA_sb[0][:, 2, :],
                                  in_=whidA_d[0][:, 2, :]),
            ]
            for i in range(2, nD):
                nc.sync.dma_start(out=whidA_sb[i][:], in_=whidA_d[i])
            nc.sync.dma_start(out=wB_sb[:], in_=wB_d[:])
            maskbs = []
            for b in range(BPC):
                maskb_hf = sm.tile([P, SC], FP16, tag="maskb_hf",
                                   name=f"maskb_hf{b}")
                nc.sync.dma_start(out=maskb_hf[:],
                                  in_=mask_d[b].to_broadcast((P, SC)))
                maskbs.append(maskb_hf)
            # encT/val are host-compacted to valid s-columns (padding zeros)
            # and marshaled partition-major, one DMA each per batch
            encT_sb = []
            val_sb = []
            for b in range(BPC):
                e_t = big.tile([P, nE, SC], FP16, tag="encT", bufs=2,
                               name=f"encT_sb{b}")
                nc.sync.dma_start(out=e_t[:], in_=encT_d[b])
                v_t = big.tile([P, nSC, TRGD], FP16, tag="val", bufs=2,
                               name=f"val_sb{b}")
                nc.sync.dma_start(out=v_t[:], in_=val_d[b])
                encT_sb.append(e_t)
                val_sb.append(v_t)

            # ---- mm1: HpT[e, t01] = sum_d W[d,e] * hidT01[d, t01] ----
            # two half-passes of 4 et each (4 psum bufs per pass).  dt-outer
            # for DMA pair-wise consumption, but the last two dt rounds go
            # et-wise with the drain right after each stop so ring slots free
            # up staggered instead of all at the end.
            HpT = big.tile([P, nE, TRG2], FP16, tag="HpT", name="HpT")
            drain_eng = [0]

            def drain(dst, src):
                if drain_eng[0] % 2 == 0:
                    nc.vector.tensor_copy(dst, src)
                else:
                    nc.scalar.copy(dst, src)
                drain_eng[0] += 1

            def w_ap(dt, et):
                if et < 4:
                    return whidA_sb[dt][:, 0, et * P:(et + 1) * P]
                return wB_sb[:, dt, (et - 4) * P:(et - 3) * P]

            def mm1_mm(pp, dt, et):
                for h in range(2):
                    hs = slice(h * 512, (h + 1) * 512)
                    chain(nc.tensor.matmul(
                        pp[:, hs], w_ap(dt, et),
                        whidA_sb[dt][:, 1 + h, :],
                        start=(dt == 0), stop=(dt == nD - 1)))

            def emit_mm1_pass(ets, warm=False):
                pps = [psp.tile([P, TRG2], F32, tag="ps", name=f"mm1ps{et}")
                       for et in ets]
                if warm:
                    # junk transposes of the identity: keep the PE busy
                    # during the DMA/preamble dead time so the HAM clock
                    # gate is warm (2.4GHz) when real work arrives, sized
                    # to end right as whid[0]'s first half lands (~10.2us).
                    # An idle gap here is doubly bad: the wait itself plus
                    # ~11 cold 427ns matmuls after it (v5 measured).  The
                    # garbage psum is overwritten by mm1's start=True.
                    # The first 14 junk ops are hoisted BEFORE the entry
                    # barrier's release-wait (after PE's arrival DRAIN):
                    # PE-busy then starts ~6.2us (gated by the hoisted scr
                    # memset) and the HAM warm transition (~busy+3.4us)
                    # fires BEFORE mm1's first matmul — every trace without
                    # this showed 5-12 cold 427ns real matmuls.  Once warm,
                    # a short data-wait gap no longer costs double (the
                    # re-throttle MID window is 3.4us).
                    junk_view = pps[0][:].bitcast(FP16)
                    for _ in range(33):
                        junk_mms.append(chain(nc.tensor.matmul(
                            junk_view[:, 0:P], scr[:], scr[:],
                            is_transpose=True, skip_group_check=True)))
                    # supply-ordered head: round0-h0 (whidA[0]'s first
                    # 2/3rds), then ALL of round 1 (whidA[1], issued 2nd),
                    # then round0-h1 (hid0-h1, issued 3rd).  f32 psum
                    # accumulation is order-independent; bank B's group is
                    # opened (start=True) by dt1-h1 since it now runs first.
                    h0, h1 = slice(0, 512), slice(512, 1024)
                    for i, et in enumerate(ets):
                        chain(nc.tensor.matmul(
                            pps[i][:, h0], w_ap(0, et),
                            whidA_sb[0][:, 1, :], start=True, stop=False))
                    for i, et in enumerate(ets):
                        chain(nc.tensor.matmul(
                            pps[i][:, h0], w_ap(1, et),
                            whidA_sb[1][:, 1, :], start=False, stop=False))
                        chain(nc.tensor.matmul(
                            pps[i][:, h1], w_ap(1, et),
                            whidA_sb[1][:, 2, :], start=True, stop=False))
                    for i, et in enumerate(ets):
                        chain(nc.tensor.matmul(
                            pps[i][:, h1], w_ap(0, et),
                            whidA_sb[0][:, 2, :], start=False, stop=False))
                    dt_start = 2
                else:
                    dt_start = 0
                for dt in range(dt_start, nD - 2):
                    for i, et in enumerate(ets):
                        mm1_mm(pps[i], dt, et)
                for i, et in enumerate(ets):
                    for dt in (nD - 2, nD - 1):
                        mm1_mm(pps[i], dt, et)
                    drain(HpT[:, et, :], pps[i][:])

            emit_mm1_pass(range(0, nE // 2), warm=True)
            emit_mm1_pass(range(nE // 2, nE))

            # ---- mm2 + masked softmax over 8 supertiles (b, tt) ----
            tiles = [(b, tt) for b in range(BPC) for tt in range(nT)]
            pks = []
            attnTs = {}

            def emit_mm2(k):
                b, tt = tiles[k]
                ts = slice(b * TRG + tt * P, b * TRG + (tt + 1) * P)
                en_ps = psp.tile([P, SC], F32, tag="ps", name=f"en{b}{tt}")
                for et in range(nE):
                    for c0, c1 in sc_chunks:
                        chain(nc.tensor.matmul(en_ps[:, c0:c1],
                                               HpT[:, et, ts],
                                               encT_sb[b][:, et, c0:c1],
                                               start=(et == 0),
                                               stop=(et == nE - 1)))
                return en_ps

            def emit_softmax(k, en_ps):
                b, tt = tiles[k]
                maskb_hf = maskbs[b]
                # packed [ae | attn] tile: one output DMA per supertile
                pk = sm.tile([P, 2 * SC], FP16, tag="aeaw", bufs=7,
                             name=f"aeaw{b}{tt}")
                nc.scalar.copy(pk[:, :SC], en_ps[:])
                negm = sm.tile([P, 1], F32, tag="negm")
                nc.vector.tensor_reduce(negm[:], en_ps[:], axis=AXL.X,
                                        op=ALU.max, negate=True)
                ex = sm.tile([P, SC], FP16, tag="ex")
                nc.scalar.activation(ex[:], en_ps[:], ACT_EXP, bias=negm[:],
                                     scale=1.0)
                rowsum = sm.tile([P, 1], F32, tag="rowsum")
                nc.vector.scalar_tensor_tensor(ex[:], ex[:], 1.0,
                                               maskb_hf[:],
                                               op0=ALU.mult, op1=ALU.mult,
                                               accum_out=rowsum[:])
                z = sm.tile([P, 1], F32, tag="z")
                nc.vector.tensor_scalar_add(z[:], rowsum[:], 1e-6)
                rz = sm.tile([P, 1], F32, tag="rz")
                nc.vector.reciprocal(rz[:], z[:])
                nc.vector.tensor_scalar_mul(pk[:, SC:], ex[:], rz[:])
                nc.sync.dma_start(out=aeaw_d[b, tt * P:(tt + 1) * P, :],
                                  in_=pk[:])
                pks.append(pk)

            def emit_tr(k):
                # PE transposes: the DMA-xbar alternative measures ~5-6us
                # per [128,1024] tile on hardware and serializes — PE does
                # all of them in well under 1us
                attn = pks[k][:, SC:]
                trp = psp.tile([P, SC], F32, tag="ps", name=f"tr{k}")
                trh = trp[:].bitcast(FP16)
                for st in range(nSC):
                    chain(nc.tensor.transpose(trh[:, st * P:(st + 1) * P],
                                              attn[:, st * P:(st + 1) * P],
                                              identh[:]))
                attnT = sm.tile([P, nSC, P], FP16, tag="attnT",
                                name=f"attnT{k}")
                nc.scalar.copy(attnT[:], trh[:, :SC])
                attnTs[k] = attnT

            for k in range(len(tiles)):
                if k == len(tiles) - 1:
                    # tr(T0) goes BEFORE mm2(T7) on the PE: its attnT copy
                    # (which also sits ahead of T7's softmax in the Scalar
                    # FIFO) then completes under mm2(T7)'s ~3us, so mm3(T0)
                    # starts with zero gap at the B->C boundary
                    emit_tr(0)
                en_ps = emit_mm2(k)
                emit_softmax(k, en_ps)

            def emit_mm3(k):
                b, tt = tiles[k]
                attnT = attnTs.pop(k)
                last = (k == len(tiles) - 1)
                ctx_sb = sm.tile([P, TRGD], FP16, tag="ctx_sb", bufs=3)
                rows = slice(tt * P, (tt + 1) * P)
                if last:
                    # shrinking chunks in 4 SEPARATE single-bank psum ring
                    # tiles (chunks sharing a tile serialize: the tile
                    # framework's write-after-read tracking made chunk c+1's
                    # matmuls wait for chunk c's drain — v5 measured 1.9us of
                    # stalls).  Each chunk drains and ships while the next
                    # multiplies in a different bank/tile, so the end-of-
                    # kernel tail is one 128-wide DVE copy + tiny DMA.
                    edges = [0, 384, 640, 896, 1024]
                    for c in range(4):
                        tp = psp.tile([P, 512], F32, tag="ps",
                                      name=f"ctx7_{c}")
                        wdt = edges[c + 1] - edges[c]
                        ps_sl = slice(0, wdt)
                        out_sl = slice(edges[c], edges[c + 1])
                        for st in range(nSC):
                            chain(nc.tensor.matmul(tp[:, ps_sl],
                                                   attnT[:, st, :],
                                                   val_sb[b][:, st, out_sl],
                                                   start=(st == 0),
                                                   stop=(st == nSC - 1)))
                        nc.vector.tensor_copy(ctx_sb[:, out_sl],
                                              tp[:, ps_sl])
                        if c < 3:
                            nc.sync.dma_start(out=ctx_d[b, rows, out_sl],
                                              in_=ctx_sb[:, out_sl])
                        else:
                            nc.scalar.dma_start(out=ctx_d[b, rows, out_sl],
                                                in_=ctx_sb[:, out_sl])
                    return
                ctx_ps = psp.tile([P, TRGD], F32, tag="ps", name=f"ctx{k}")
                for st in range(nSC):
                    for h in range(2):
                        hs = slice(h * 512, (h + 1) * 512)
                        chain(nc.tensor.matmul(ctx_ps[:, hs],
                                               attnT[:, st, :],
                                               val_sb[b][:, st, hs],
                                               start=(st == 0),
                                               stop=(st == nSC - 1)))
                # DVE, not Scalar: the Scalar FIFO still holds late
                # attn-transpose issues during early mm3 tiles
                nc.vector.tensor_copy(ctx_sb[:], ctx_ps[:])
                nc.sync.dma_start(out=ctx_d[b, rows, :], in_=ctx_sb[:])

            # transposes run two tiles ahead of mm3: each attnT Scalar copy
            # then has two full mm3 rounds of cover before its consumer
            emit_tr(1)
            for k in range(len(tiles)):
                if k + 2 < len(tiles):
                    emit_tr(k + 2)
                emit_mm3(k)

    # ---- entry-barrier hoist ----
    # The Bacc-init all-engine barrier gates the first DMA issue at ~7.2us
    # into the graded window while Sync sits idle from ~5.9us.  Hoist the
    # three supply-critical DMA issues to between Sync's barrier-arrival
    # DRAIN and its barrier-release wait (after the drain, so the barrier
    # never waits on the transfers), and the junk-scratch memset likewise on
    # DVE.  Semaphores are zeroed by the NEFF preamble before any module
    # instruction, and each hoisted instruction carries its sync_info with
    # it; instructions with semaphore waits are left in place.
    il0 = nc.main_func.blocks[0].instructions

    def hoist_before(inst, anchor_name_prefix):
        # Move a body-block instruction into the init block, right before
        # its engine's entry-barrier release-wait (and hence after that
        # engine's barrier-arrival DRAIN — a DRAIN waits on the engine's
        # outstanding DMAs, so issuing before it would stall the barrier).
        dbg = os.environ.get("KERNEL_HOIST_DEBUG")
        si = inst.sync_info
        if si is not None and len(si.on_wait) > 0:
            if dbg:
                print(f"HOIST-SKIP waits {inst.name}: {si.on_wait}")
            return
        src = None
        for b in nc.main_func.blocks:
            if inst in b.instructions:
                src = b
                break
        if src is None:
            if dbg:
                print(f"HOIST-SKIP notfound {inst.name}")
            return
        for k, x in enumerate(il0):
            if type(x).__name__ == "InstEventSemaphore" and str(
                    x.name).startswith(anchor_name_prefix):
                src.instructions.remove(inst)
                il0.insert(k, inst)
                if dbg:
                    print(f"HOIST-OK {inst.name} -> before {x.name} @{k}")
                return
        if dbg:
            print(f"HOIST-SKIP noanchor {inst.name}")

    hoist_before(scr_set.ins, "barrier_DVE_")
    for d in early_dmas:
        hoist_before(d.ins, "barrier_SP_")
    for j in junk_mms[:14]:
        hoist_before(j.ins, "barrier_PE_")

    nc.compile()
    return nc


def kernel(hidden, encoder_outputs, encoder_value, encoder_mask, W):
    global LAST_EXEC_NS, LAST_RESULTS
    from concourse.bass_utils import run_bass_kernel_spmd

    hidden = np.ascontiguousarray(hidden, dtype=np.float32)
    encoder_outputs = np.ascontiguousarray(encoder_outputs, dtype=np.float32)
    encoder_value = np.ascontiguousarray(encoder_value, dtype=np.float32)
    encoder_mask = np.ascontiguousarray(encoder_mask, dtype=np.float32)
    W = np.ascontiguousarray(W, dtype=np.float32)

    # s-compaction: gather valid source positions per batch; SC is the max
    # valid count rounded up to 128 (uniform across cores — SPMD shares one
    # compiled program)
    idx = [np.nonzero(encoder_mask[b] > 0.5)[0] for b in range(B)]
    n_max = max(len(ix) for ix in idx)
    SC = min(SRC, -(-max(n_max, 1) // P) * P)
    nSC = SC // P

    if ("nc", SC) not in _cache:
        _cache[("nc", SC)] = _build(SC)
    nc = _cache[("nc", SC)]

    w_tiles = W.astype(np.float16).reshape(nD, P, ENCD)
    # (P, nD, 512) to match the SBUF tile's linear order — DMA does not
    # transpose
    wB = np.ascontiguousarray(w_tiles[:, :, 512:].transpose(1, 0, 2))
    in_maps = []
    for c in range(NCORES):
        sl = slice(c * BPC, (c + 1) * BPC)
        hid2 = hidden[sl]  # (2, TRG, TRGD)
        hidT01 = np.concatenate([hid2[0].T, hid2[1].T], axis=1)
        hid_tiles = hidT01.astype(np.float16).reshape(nD, P, TRG2)
        whidA = np.empty((nD, P, 3, 512), dtype=np.float16)
        whidA[:, :, 0, :] = w_tiles[:, :, 0:512]
        whidA[:, :, 1, :] = hid_tiles[:, :, 0:512]
        whidA[:, :, 2, :] = hid_tiles[:, :, 512:]
        encT_pm = np.zeros((BPC, P, nE, SC), dtype=np.float16)
        val_pm = np.zeros((BPC, P, nSC, TRGD), dtype=np.float16)
        maskc = np.zeros((BPC, 1, SC), dtype=np.float16)
        for j in range(BPC):
            b = c * BPC + j
            ix = idx[b]
            n_b = len(ix)
            # encT compact: [ENCD, n_b] -> pad to SC -> partition-major
            encTc = encoder_outputs[b].T[:, ix].astype(np.float16)
            encT_pm[j, :, :, :n_b] = encTc.reshape(nE, P, n_b).transpose(
                1, 0, 2)
            # val compact: [n_b, TRGD] -> pad to SC rows -> partition-major
            valc = np.zeros((SC, TRGD), dtype=np.float16)
            valc[:n_b] = encoder_value[b][ix]
            val_pm[j] = valc.reshape(nSC, P, TRGD).transpose(1, 0, 2)
            maskc[j, 0, :n_b] = 1.0
        in_maps.append({
            "whidA": whidA,
            "wB": wB,
            "encT": encT_pm,
            "val": val_pm,
            "mask": maskc,
        })

    trace = bool(int(os.environ.get("KERNEL_TRACE", "0")))
    res = run_bass_kernel_spmd(nc, in_maps, core_ids=list(range(NCORES)),
                               trace=trace)
    LAST_EXEC_NS = res.exec_time_ns
    LAST_RESULTS = res

    context = np.concatenate([res.results[c]["ctx"] for c in range(NCORES)],
                             axis=0).astype(np.float32)
    # scatter compacted ae/aw back to full source width; masked positions
    # are exact zeros in the reference (energies*mask and e*mask)
    attn_energies = np.zeros((B, TRG, SRC), dtype=np.float32)
    attn_weights = np.zeros((B, TRG, SRC), dtype=np.float32)
    for c in range(NCORES):
        aeaw = res.results[c]["aeaw"]
        for j in range(BPC):
            b = c * BPC + j
            ix = idx[b]
            n_b = len(ix)
            attn_energies[b][:, ix] = aeaw[j][:, :n_b].astype(np.float32)
            attn_weights[b][:, ix] = aeaw[j][:, SC:SC + n_b].astype(
                np.float32)
    return context, attn_weights, attn_energies


# revision 43
# speedup vs baseline: 1.0844x; 1.0192x over previous
"""Trainium2 Bass kernel for nn_Attention (general-score attention with
masked softmax), data-parallel over batch across 8 NeuronCores.

Math (per batch), matching the reference exactly for {0,1} float masks:
    raw[t,s]  = sum_e (hidden @ W)[t,e] * enc[s,e]       (associativity trick:
                (hidden @ W) @ enc^T  ==  hidden @ (enc @ W^T)^T, saves 25%
                FLOPs and avoids materializing proj)
    attn_energies = raw * mask            (mask in {0,1} so mask^2 == mask)
    e = exp(x - max_s x) * mask
    attn = e / (sum_s e + 1e-6)
    context = attn @ enc_value

v10 (95.3us), built up from ntff analysis of v3 (105.9us) through v9.
Measured facts this version is built on: 512-wide fp16 matmuls issue
back-to-back at 216ns warm (LDWEIGHTS hides); Sync HW-DGE dma_start
issue costs ~0.68us serial; DMA completion-to-semaphore lag ~0.8us;
GpSimd SW-DGE issues cost ~5us (never use); a parallel Scalar-queue
prefetch starves the critical stream at the HBM; the graded window runs
from the first module instruction to the last framework-epilogue
instruction (the ~7.7us full-semaphore-file zeroing is a fixed tax);
fp8 e4m3 measures 3.7% matmul rel err (budget 2e-2) so fp16 is the
fastest legal dtype; occasional runs land in P0 power state (PE at
2.0GHz, 259ns/MM) — rerun after a cooldown to compare fairly.

Changes vs v3:
  - s-compaction: the {0,1} source mask keeps ~86% of the 1024 source
    positions.  The host gathers the valid s-columns per batch (encT, val)
    into a compact SC-wide layout (SC = max valid count over batches,
    rounded up to 128; 896 for the graded inputs), the kernel computes
    energies/softmax/ctx over SC columns only, and the host scatters the
    ae/aw outputs back to full width with exact zeros at masked positions
    (the reference's masked entries are exact zeros).  Saves 12.5% of the
    mm2 streaming cycles and 1/8 of the mm3 matmuls + transposes.  The row
    max over the compact tile equals the reference's max over x*mask
    exactly (padded columns give raw energy 0 = the reference's masked 0).
  - (w-pass1-half, hidT) pairs fused into single 0.375MB DMAs via a
    host-interleaved whidA[(dt, p, 3, 512)] layout; pass-2's w halves
    ride as ONE fused 1MB tensor.  mm1's supply-critical stream is 25%
    lighter and needs 10 Sync issues instead of 16.  All loads stay on
    Sync in consumption order (supply order whidA0a, whidA1, whidA0b
    matches the reordered round-0-h0 / round-1 / round-0-h1 consumption;
    f32 psum accumulation is order-independent).
  - entry-barrier hoist: the three supply-critical DMA issues (and the
    junk-scratch memset, on DVE) are relocated between their engine's
    barrier-arrival DRAIN and barrier-release wait, so the first DMA
    issues at ~6.6us instead of ~7.2us and mm1 starts at ~9.7us.
  - junk HAM-warmup transposes sized (28) to end right as the first pair's
    data+semaphore land; an idle gap before mm1 is doubly bad (the wait
    itself plus ~12 cold 427ns matmuls, HAM's warm transition being
    absolute-time).  The Scalar HW-DGE queue measured a ~4us first-use
    cold-start, so supply-critical loads must stay on Sync's (preamble-
    warmed) queue.
  - last ctx tile drains in shrinking chunks (384/256/256/128) in four
    SEPARATE single-bank psum ring tiles — chunks sharing a tile
    serialize on the tile framework's write-after-read tracking (v5
    measured 1.9us of stalls); the end-of-kernel tail is one 128-wide
    DVE copy + tiny DMA.  ctx_sb ring deepened to 3.
  - everything else as v3: fp16 gemms with f32 PSUM accumulation, fused
    two-batch mm1 with dt-outer DMA-paired consumption and staggered
    et-wise drains, PE-transposes of attn two tiles ahead of mm3, packed
    [ae|aw] output tiles.
"""
import os

import numpy as np

B, TRG, SRC, ENCD, TRGD = 16, 512, 1024, 1024, 1024
NCORES = 8
BPC = B // NCORES  # batches per core
P = 128
nD = TRGD // P   # 8 contraction tiles over d
nE = ENCD // P   # 8 over e
nT = TRG // P    # 4 t-tiles per batch
TRG2 = BPC * TRG  # both batches fused along t: 1024

_cache = {}

LAST_EXEC_NS = None
LAST_RESULTS = None


def _build(SC):
    import bass_rust
    import concourse.mybir as mybir
    import concourse.tile as tile
    from concourse import bacc
    from concourse.masks import make_identity

    _add_dep = bass_rust.add_dep_helper

    F32 = mybir.dt.float32
    FP16 = mybir.dt.float16
    ALU = mybir.AluOpType
    AXL = mybir.AxisListType
    ACT_EXP = mybir.ActivationFunctionType.Exp

    nSC = SC // P  # compacted source tiles
    # mm2 moving-operand chunks over the SC free dim (PSUM banks are 512
    # f32, so chunk boundaries stay bank-aligned at 512)
    sc_chunks = []
    off = 0
    while off < SC:
        w = min(512, SC - off)
        sc_chunks.append((off, off + w))
        off += w

    nc = bacc.Bacc("TRN2", target_bir_lowering=False, debug=False)

    whidA_d = nc.dram_tensor("whidA", (nD, P, 3, 512), FP16,
                             kind="ExternalInput")
    wB_d = nc.dram_tensor("wB", (P, nD, 512), FP16, kind="ExternalInput")
    encT_d = nc.dram_tensor("encT", (BPC, P, nE, SC), FP16,
                            kind="ExternalInput")
    val_d = nc.dram_tensor("val", (BPC, P, nSC, TRGD), FP16,
                           kind="ExternalInput")
    mask_d = nc.dram_tensor("mask", (BPC, 1, SC), FP16, kind="ExternalInput")
    aeaw_d = nc.dram_tensor("aeaw", (BPC, TRG, 2 * SC), FP16,
                            kind="ExternalOutput")
    ctx_d = nc.dram_tensor("ctx", (BPC, TRG, TRGD), FP16,
                           kind="ExternalOutput")

    with tile.TileContext(nc) as tc:
        with (
            tc.tile_pool(name="const", bufs=1) as const,
            tc.tile_pool(name="big", bufs=1) as big,
            tc.tile_pool(name="sm", bufs=2) as sm,
            tc.tile_pool(name="ps", bufs=4, space="PSUM") as psp,
        ):
            # junk-warmup scratch: a bare memset is ready ~1.3us before the
            # identity (memset+affine_select+copy), so the HAM warmup can
            # start that much sooner.  On DVE so it can be hoisted before
            # the entry barrier (see the block surgery below).
            scr = const.tile([P, P], FP16)
            scr_set = nc.vector.memset(scr[:], 0.0)
            ident = const.tile([P, P], F32)
            make_identity(nc, ident[:])
            identh = const.tile([P, P], FP16)
            nc.vector.tensor_copy(identh[:], ident[:])

            # PE program order is pinned with an explicit linear chain so the
            # scheduler can never interleave accumulation groups or delay a
            # group's stop.
            pe_prev = [None]

            def chain(mm):
                if pe_prev[0] is not None:
                    _add_dep(mm.ins, pe_prev[0].ins, sync=False,
                             reason="pe order")
                pe_prev[0] = mm
                return mm

            # ---- loads (issue order == consumption order) ----
            # whidA[dt] carries w[dt] cols 0:512 (all pass-1 needs) plus the
            # full hidT[dt]; pass-2's w halves ride later as ONE fused 1MB
            # DMA.  This thins mm1's supply-critical stream by 25% and cuts
            # Sync's serial issue count (measured ~0.68us per dma_start,
            # ~0.8us DMA-completion-to-semaphore lag).
            whidA_sb = [big.tile([P, 3, 512], FP16, tag=f"whidA{i}",
                                 name=f"whidA_sb{i}") for i in range(nD)]
            wB_sb = big.tile([P, nD, 512], FP16, tag="wB", name="wB_sb")
            # whidA[0] split: (w0-half + hid0-h0) first so round0-h0 starts
            # on 0.25MB; hid0-h1 follows whidA[1] in supply order to match
            # the reordered round-0/round-1 consumption below
            # All loads on Sync's HW-DGE in consumption order.  The Scalar
            # queue (Q10) measured a ~4us first-use cold-start lag, so it
            # must NOT carry supply-critical loads; Sync's queue is warmed
            # by the framework preamble.  The first three issues are
            # hoisted before the entry barrier's release-wait (Sync's
            # barrier-arrival drain completes ~6.2us, ~0.4us before the
            # release fires).
            early_dmas = [
                nc.sync.dma_start(out=whidA_sb[0][:, 0:2, :],
                                  in_=whidA_d[0][:, 0:2, :]),
                nc.sync.dma_start(out=whidA_sb[1][:], in_=whidA_d[1]),
                nc.sync.dma_start(out=whidA_sb[0][:, 2, :],
                                  in_=whidA_d[0][:, 2, :]),
            ]
            for i in range(2, nD):
                nc.sync.dma_start(out=whidA_sb[i][:], in_=whidA_d[i])
            nc.sync.dma_start(out=wB_sb[:], in_=wB_d[:])
            maskbs = []
            for b in range(BPC):
                maskb_hf = sm.tile([P, SC], FP16, tag="maskb_hf",
                                   name=f"maskb_hf{b}")
                nc.sync.dma_start(out=maskb_hf[:],
                                  in_=mask_d[b].to_broadcast((P, SC)))
                maskbs.append(maskb_hf)
            # encT/val are host-compacted to valid s-columns (padding zeros)
            # and marshaled partition-major, one DMA each per batch
            encT_sb = []
            val_sb = []
            for b in range(BPC):
                e_t = big.tile([P, nE, SC], FP16, tag="encT", bufs=2,
                               name=f"encT_sb{b}")
                nc.sync.dma_start(out=e_t[:], in_=encT_d[b])
                v_t = big.tile([P, nSC, TRGD], FP16, tag="val", bufs=2,
                               name=f"val_sb{b}")
                nc.sync.dma_start(out=v_t[:], in_=val_d[b])
                encT_sb.append(e_t)
                val_sb.append(v_t)

            # ---- mm1: HpT[e, t01] = sum_d W[d,e] * hidT01[d, t01] ----
            # two half-passes of 4 et each (4 psum bufs per pass).  dt-outer
            # for DMA pair-wise consumption, but the last two dt rounds go
            # et-wise with the drain right after each stop so ring slots free
            # up staggered instead of all at the end.
            HpT = big.tile([P, nE, TRG2], FP16, tag="HpT", name="HpT")
            drain_eng = [0]

            def drain(dst, src):
                if drain_eng[0] % 2 == 0:
                    nc.vector.tensor_copy(dst, src)
                else:
                    nc.scalar.copy(dst, src)
                drain_eng[0] += 1

            def w_ap(dt, et):
                if et < 4:
                    return whidA_sb[dt][:, 0, et * P:(et + 1) * P]
                return wB_sb[:, dt, (et - 4) * P:(et - 3) * P]

            def mm1_mm(pp, dt, et):
                for h in range(2):
                    hs = slice(h * 512, (h + 1) * 512)
                    chain(nc.tensor.matmul(
                        pp[:, hs], w_ap(dt, et),
                        whidA_sb[dt][:, 1 + h, :],
                        start=(dt == 0), stop=(dt == nD - 1)))

            def emit_mm1_pass(ets, warm=False):
                pps = [psp.tile([P, TRG2], F32, tag="ps", name=f"mm1ps{et}")
                       for et in ets]
                if warm:
                    # junk transposes of the identity: keep the PE busy
                    # during the DMA/preamble dead time so the HAM clock
                    # gate is warm (2.4GHz) when real work arrives, sized
                    # to end right as whid[0]'s first half lands (~10.2us).
                    # An idle gap here is doubly bad: the wait itself plus
                    # ~11 cold 427ns matmuls after it (v5 measured).  The
                    # garbage psum is overwritten by mm1's start=True.
                    junk_view = pps[0][:].bitcast(FP16)
                    for _ in range(28):
                        chain(nc.tensor.matmul(
                            junk_view[:, 0:P], scr[:], scr[:],
                            is_transpose=True, skip_group_check=True))
                    # supply-ordered head: round0-h0 (whidA[0]'s first
                    # 2/3rds), then ALL of round 1 (whidA[1], issued 2nd),
                    # then round0-h1 (hid0-h1, issued 3rd).  f32 psum
                    # accumulation is order-independent; bank B's group is
                    # opened (start=True) by dt1-h1 since it now runs first.
                    h0, h1 = slice(0, 512), slice(512, 1024)
                    for i, et in enumerate(ets):
                        chain(nc.tensor.matmul(
                            pps[i][:, h0], w_ap(0, et),
                            whidA_sb[0][:, 1, :], start=True, stop=False))
                    for i, et in enumerate(ets):
                        chain(nc.tensor.matmul(
                            pps[i][:, h0], w_ap(1, et),
                            whidA_sb[1][:, 1, :], start=False, stop=False))
                        chain(nc.tensor.matmul(
                            pps[i][:, h1], w_ap(1, et),
                            whidA_sb[1][:, 2, :], start=True, stop=False))
                    for i, et in enumerate(ets):
                        chain(nc.tensor.matmul(
                            pps[i][:, h1], w_ap(0, et),
                            whidA_sb[0][:, 2, :], start=False, stop=False))
                    dt_start = 2
                else:
                    dt_start = 0
                for dt in range(dt_start, nD - 2):
                    for i, et in enumerate(ets):
                        mm1_mm(pps[i], dt, et)
                for i, et in enumerate(ets):
                    for dt in (nD - 2, nD - 1):
                        mm1_mm(pps[i], dt, et)
                    drain(HpT[:, et, :], pps[i][:])

            emit_mm1_pass(range(0, nE // 2), warm=True)
            emit_mm1_pass(range(nE // 2, nE))

            # ---- mm2 + masked softmax over 8 supertiles (b, tt) ----
            tiles = [(b, tt) for b in range(BPC) for tt in range(nT)]
            pks = []
            attnTs = {}

            def emit_mm2(k):
                b, tt = tiles[k]
                ts = slice(b * TRG + tt * P, b * TRG + (tt + 1) * P)
                en_ps = psp.tile([P, SC], F32, tag="ps", name=f"en{b}{tt}")
                for et in range(nE):
                    for c0, c1 in sc_chunks:
                        chain(nc.tensor.matmul(en_ps[:, c0:c1],
                                               HpT[:, et, ts],
                                               encT_sb[b][:, et, c0:c1],
                                               start=(et == 0),
                                               stop=(et == nE - 1)))
                return en_ps

            def emit_softmax(k, en_ps):
                b, tt = tiles[k]
                maskb_hf = maskbs[b]
                # packed [ae | attn] tile: one output DMA per supertile
                pk = sm.tile([P, 2 * SC], FP16, tag="aeaw", bufs=7,
                             name=f"aeaw{b}{tt}")
                nc.scalar.copy(pk[:, :SC], en_ps[:])
                negm = sm.tile([P, 1], F32, tag="negm")
                nc.vector.tensor_reduce(negm[:], en_ps[:], axis=AXL.X,
                                        op=ALU.max, negate=True)
                ex = sm.tile([P, SC], FP16, tag="ex")
                nc.scalar.activation(ex[:], en_ps[:], ACT_EXP, bias=negm[:],
                                     scale=1.0)
                rowsum = sm.tile([P, 1], F32, tag="rowsum")
                nc.vector.scalar_tensor_tensor(ex[:], ex[:], 1.0,
                                               maskb_hf[:],
                                               op0=ALU.mult, op1=ALU.mult,
                                               accum_out=rowsum[:])
                z = sm.tile([P, 1], F32, tag="z")
                nc.vector.tensor_scalar_add(z[:], rowsum[:], 1e-6)
                rz = sm.tile([P, 1], F32, tag="rz")
                nc.vector.reciprocal(rz[:], z[:])
                nc.vector.tensor_scalar_mul(pk[:, SC:], ex[:], rz[:])
                nc.sync.dma_start(out=aeaw_d[b, tt * P:(tt + 1) * P, :],
                                  in_=pk[:])
                pks.append(pk)

            def emit_tr(k):
                # PE transposes: the DMA-xbar alternative measures ~5-6us
                # per [128,1024] tile on hardware and serializes — PE does
                # all of them in well under 1us
                attn = pks[k][:, SC:]
                trp = psp.tile([P, SC], F32, tag="ps", name=f"tr{k}")
                trh = trp[:].bitcast(FP16)
                for st in range(nSC):
                    chain(nc.tensor.transpose(trh[:, st * P:(st + 1) * P],
                                              attn[:, st * P:(st + 1) * P],
                                              identh[:]))
                # ring of 3: with 2, each attnT Scalar copy waits for
                # mm3(k-2)'s last matmul to release its slot, leaving zero
                # margin at the C-phase tail; 3 buys one extra mm3 round
                attnT = sm.tile([P, nSC, P], FP16, tag="attnT", bufs=3,
                                name=f"attnT{k}")
                nc.scalar.copy(attnT[:], trh[:, :SC])
                attnTs[k] = attnT

            for k in range(len(tiles)):
                if k == len(tiles) - 1:
                    # tr(T0) goes BEFORE mm2(T7) on the PE: its attnT copy
                    # (which also sits ahead of T7's softmax in the Scalar
                    # FIFO) then completes under mm2(T7)'s ~3us, so mm3(T0)
                    # starts with zero gap at the B->C boundary
                    emit_tr(0)
                en_ps = emit_mm2(k)
                emit_softmax(k, en_ps)

            def emit_mm3(k):
                b, tt = tiles[k]
                attnT = attnTs.pop(k)
                last = (k == len(tiles) - 1)
                ctx_sb = sm.tile([P, TRGD], FP16, tag="ctx_sb", bufs=3)
                rows = slice(tt * P, (tt + 1) * P)
                if last:
                    # shrinking chunks in 4 SEPARATE single-bank psum ring
                    # tiles (chunks sharing a tile serialize: the tile
                    # framework's write-after-read tracking made chunk c+1's
                    # matmuls wait for chunk c's drain — v5 measured 1.9us of
                    # stalls).  Each chunk drains and ships while the next
                    # multiplies in a different bank/tile, so the end-of-
                    # kernel tail is one 128-wide DVE copy + tiny DMA.
                    edges = [0, 384, 640, 896, 1024]
                    for c in range(4):
                        tp = psp.tile([P, 512], F32, tag="ps",
                                      name=f"ctx7_{c}")
                        wdt = edges[c + 1] - edges[c]
                        ps_sl = slice(0, wdt)
                        out_sl = slice(edges[c], edges[c + 1])
                        for st in range(nSC):
                            chain(nc.tensor.matmul(tp[:, ps_sl],
                                                   attnT[:, st, :],
                                                   val_sb[b][:, st, out_sl],
                                                   start=(st == 0),
                                                   stop=(st == nSC - 1)))
                        nc.vector.tensor_copy(ctx_sb[:, out_sl],
                                              tp[:, ps_sl])
                        if c < 3:
                            nc.sync.dma_start(out=ctx_d[b, rows, out_sl],
                                              in_=ctx_sb[:, out_sl])
                        else:
                            nc.scalar.dma_start(out=ctx_d[b, rows, out_sl],
                                                in_=ctx_sb[:, out_sl])
                    return
                ctx_ps = psp.tile([P, TRGD], F32, tag="ps", name=f"ctx{k}")
                for st in range(nSC):
                    for h in range(2):
                        hs = slice(h * 512, (h + 1) * 512)
                        chain(nc.tensor.matmul(ctx_ps[:, hs],
                                               attnT[:, st, :],
                                               val_sb[b][:, st, hs],
                                               start=(st == 0),
                                               stop=(st == nSC - 1)))
                # DVE, not Scalar: the Scalar FIFO still holds late
                # attn-transpose issues during early mm3 tiles
                nc.vector.tensor_copy(ctx_sb[:], ctx_ps[:])
                nc.sync.dma_start(out=ctx_d[b, rows, :], in_=ctx_sb[:])

            # transposes run two tiles ahead of mm3: each attnT Scalar copy
            # then has two full mm3 rounds of cover before its consumer
            emit_tr(1)
            for k in range(len(tiles)):
                if k + 2 < len(tiles):
                    emit_tr(k + 2)
                emit_mm3(k)

    # ---- entry-barrier hoist ----
    # The Bacc-init all-engine barrier gates the first DMA issue at ~7.2us
    # into the graded window while Sync sits idle from ~5.9us.  Hoist the
    # three supply-critical DMA issues to between Sync's barrier-arrival
    # DRAIN and its barrier-release wait (after the drain, so the barrier
    # never waits on the transfers), and the junk-scratch memset likewise on
    # DVE.  Semaphores are zeroed by the NEFF preamble before any module
    # instruction, and each hoisted instruction carries its sync_info with
    # it; instructions with semaphore waits are left in place.
    il0 = nc.main_func.blocks[0].instructions

    def hoist_before(inst, anchor_name_prefix):
        # Move a body-block instruction into the init block, right before
        # its engine's entry-barrier release-wait (and hence after that
        # engine's barrier-arrival DRAIN — a DRAIN waits on the engine's
        # outstanding DMAs, so issuing before it would stall the barrier).
        dbg = os.environ.get("KERNEL_HOIST_DEBUG")
        si = inst.sync_info
        if si is not None and len(si.on_wait) > 0:
            if dbg:
                print(f"HOIST-SKIP waits {inst.name}: {si.on_wait}")
            return
        src = None
        for b in nc.main_func.blocks:
            if inst in b.instructions:
                src = b
                break
        if src is None:
            if dbg:
                print(f"HOIST-SKIP notfound {inst.name}")
            return
        for k, x in enumerate(il0):
            if type(x).__name__ == "InstEventSemaphore" and str(
                    x.name).startswith(anchor_name_prefix):
                src.instructions.remove(inst)
                il0.insert(k, inst)
                if dbg:
                    print(f"HOIST-OK {inst.name} -> before {x.name} @{k}")
                return
        if dbg:
            print(f"HOIST-SKIP noanchor {inst.name}")

    hoist_before(scr_set.ins, "barrier_DVE_")
    for d in early_dmas:
        hoist_before(d.ins, "barrier_SP_")

    nc.compile()
    return nc


def kernel(hidden, encoder_outputs, encoder_value, encoder_mask, W):
    global LAST_EXEC_NS, LAST_RESULTS
    from concourse.bass_utils import run_bass_kernel_spmd

    hidden = np.ascontiguousarray(hidden, dtype=np.float32)
    encoder_outputs = np.ascontiguousarray(encoder_outputs, dtype=np.float32)
    encoder_value = np.ascontiguousarray(encoder_value, dtype=np.float32)
    encoder_mask = np.ascontiguousarray(encoder_mask, dtype=np.float32)
    W = np.ascontiguousarray(W, dtype=np.float32)

    # s-compaction: gather valid source positions per batch; SC is the max
    # valid count rounded up to 128 (uniform across cores — SPMD shares one
    # compiled program)
    idx = [np.nonzero(encoder_mask[b] > 0.5)[0] for b in range(B)]
    n_max = max(len(ix) for ix in idx)
    SC = min(SRC, -(-max(n_max, 1) // P) * P)
    nSC = SC // P

    if ("nc", SC) not in _cache:
        _cache[("nc", SC)] = _build(SC)
    nc = _cache[("nc", SC)]

    w_tiles = W.astype(np.float16).reshape(nD, P, ENCD)
    # (P, nD, 512) to match the SBUF tile's linear order — DMA does not
    # transpose
    wB = np.ascontiguousarray(w_tiles[:, :, 512:].transpose(1, 0, 2))
    in_maps = []
    for c in range(NCORES):
        sl = slice(c * BPC, (c + 1) * BPC)
        hid2 = hidden[sl]  # (2, TRG, TRGD)
        hidT01 = np.concatenate([hid2[0].T, hid2[1].T], axis=1)
        hid_tiles = hidT01.astype(np.float16).reshape(nD, P, TRG2)
        whidA = np.empty((nD, P, 3, 512), dtype=np.float16)
        whidA[:, :, 0, :] = w_tiles[:, :, 0:512]
        whidA[:, :, 1, :] = hid_tiles[:, :, 0:512]
        whidA[:, :, 2, :] = hid_tiles[:, :, 512:]
        encT_pm = np.zeros((BPC, P, nE, SC), dtype=np.float16)
        val_pm = np.zeros((BPC, P, nSC, TRGD), dtype=np.float16)
        maskc = np.zeros((BPC, 1, SC), dtype=np.float16)
        for j in range(BPC):
            b = c * BPC + j
            ix = idx[b]
            n_b = len(ix)
            # encT compact: [ENCD, n_b] -> pad to SC -> partition-major
            encTc = encoder_outputs[b].T[:, ix].astype(np.float16)
            encT_pm[j, :, :, :n_b] = encTc.reshape(nE, P, n_b).transpose(
                1, 0, 2)
            # val compact: [n_b, TRGD] -> pad to SC rows -> partition-major
            valc = np.zeros((SC, TRGD), dtype=np.float16)
            valc[:n_b] = encoder_value[b][ix]
            val_pm[j] = valc.reshape(nSC, P, TRGD).transpose(1, 0, 2)
            maskc[j, 0, :n_b] = 1.0
        in_maps.append({
            "whidA": whidA,
            "wB": wB,
            "encT": encT_pm,
            "val": val_pm,
            "mask": maskc,
        })

    trace = bool(int(os.environ.get("KERNEL_TRACE", "0")))
    res = run_bass_kernel_spmd(nc, in_maps, core_ids=list(range(NCORES)),
                               trace=trace)
    LAST_EXEC_NS = res.exec_time_ns
    LAST_RESULTS = res

    context = np.concatenate([res.results[c]["ctx"] for c in range(NCORES)],
                             axis=0).astype(np.float32)
    # scatter compacted ae/aw back to full source width; masked positions
    # are exact zeros in the reference (energies*mask and e*mask)
    attn_energies = np.zeros((B, TRG, SRC), dtype=np.float32)
    attn_weights = np.zeros((B, TRG, SRC), dtype=np.float32)
    for c in range(NCORES):
        aeaw = res.results[c]["aeaw"]
        for j in range(BPC):
            b = c * BPC + j
            ix = idx[b]
            n_b = len(ix)
            attn_energies[b][:, ix] = aeaw[j][:, :n_b].astype(np.float32)
            attn_weights[b][:, ix] = aeaw[j][:, SC:SC + n_b].astype(
                np.float32)
    return context, attn_weights, attn_energies


# revision 45
# speedup vs baseline: 1.1018x; 1.0160x over previous
"""Trainium2 Bass kernel for nn_Attention (general-score attention with
masked softmax), data-parallel over batch across 8 NeuronCores.

Math (per batch), matching the reference exactly for {0,1} float masks:
    raw[t,s]  = sum_e (hidden @ W)[t,e] * enc[s,e]       (associativity trick:
                (hidden @ W) @ enc^T  ==  hidden @ (enc @ W^T)^T, saves 25%
                FLOPs and avoids materializing proj)
    attn_energies = raw * mask            (mask in {0,1} so mask^2 == mask)
    e = exp(x - max_s x) * mask
    attn = e / (sum_s e + 1e-6)
    context = attn @ enc_value

v10 (95.3us), built up from ntff analysis of v3 (105.9us) through v9.
Measured facts this version is built on: 512-wide fp16 matmuls issue
back-to-back at 216ns warm (LDWEIGHTS hides); Sync HW-DGE dma_start
issue costs ~0.68us serial; DMA completion-to-semaphore lag ~0.8us;
GpSimd SW-DGE issues cost ~5us (never use); a parallel Scalar-queue
prefetch starves the critical stream at the HBM; the graded window runs
from the first module instruction to the last framework-epilogue
instruction (the ~7.7us full-semaphore-file zeroing is a fixed tax);
fp8 e4m3 measures 3.7% matmul rel err (budget 2e-2) so fp16 is the
fastest legal dtype; occasional runs land in P0 power state (PE at
2.0GHz, 259ns/MM) — rerun after a cooldown to compare fairly.

Changes vs v3:
  - s-compaction: the {0,1} source mask keeps ~86% of the 1024 source
    positions.  The host gathers the valid s-columns per batch (encT, val)
    into a compact SC-wide layout (SC = max valid count over batches,
    rounded up to 128; 896 for the graded inputs), the kernel computes
    energies/softmax/ctx over SC columns only, and the host scatters the
    ae/aw outputs back to full width with exact zeros at masked positions
    (the reference's masked entries are exact zeros).  Saves 12.5% of the
    mm2 streaming cycles and 1/8 of the mm3 matmuls + transposes.  The row
    max over the compact tile equals the reference's max over x*mask
    exactly (padded columns give raw energy 0 = the reference's masked 0).
  - (w-pass1-half, hidT) pairs fused into single 0.375MB DMAs via a
    host-interleaved whidA[(dt, p, 3, 512)] layout; pass-2's w halves
    ride as ONE fused 1MB tensor.  mm1's supply-critical stream is 25%
    lighter and needs 10 Sync issues instead of 16.  All loads stay on
    Sync in consumption order (supply order whidA0a, whidA1, whidA0b
    matches the reordered round-0-h0 / round-1 / round-0-h1 consumption;
    f32 psum accumulation is order-independent).
  - entry-barrier hoist: the three supply-critical DMA issues (and the
    junk-scratch memset, on DVE) are relocated between their engine's
    barrier-arrival DRAIN and barrier-release wait, so the first DMA
    issues at ~6.6us instead of ~7.2us and mm1 starts at ~9.7us.
  - junk HAM-warmup transposes sized (28) to end right as the first pair's
    data+semaphore land; an idle gap before mm1 is doubly bad (the wait
    itself plus ~12 cold 427ns matmuls, HAM's warm transition being
    absolute-time).  The Scalar HW-DGE queue measured a ~4us first-use
    cold-start, so supply-critical loads must stay on Sync's (preamble-
    warmed) queue.
  - last ctx tile drains in shrinking chunks (384/256/256/128) in four
    SEPARATE single-bank psum ring tiles — chunks sharing a tile
    serialize on the tile framework's write-after-read tracking (v5
    measured 1.9us of stalls); the end-of-kernel tail is one 128-wide
    DVE copy + tiny DMA.  ctx_sb ring deepened to 3.
  - everything else as v3: fp16 gemms with f32 PSUM accumulation, fused
    two-batch mm1 with dt-outer DMA-paired consumption and staggered
    et-wise drains, PE-transposes of attn two tiles ahead of mm3, packed
    [ae|aw] output tiles.
"""
import os

import numpy as np

B, TRG, SRC, ENCD, TRGD = 16, 512, 1024, 1024, 1024
NCORES = 8
BPC = B // NCORES  # batches per core
P = 128
nD = TRGD // P   # 8 contraction tiles over d
nE = ENCD // P   # 8 over e
nT = TRG // P    # 4 t-tiles per batch
TRG2 = BPC * TRG  # both batches fused along t: 1024

_cache = {}

LAST_EXEC_NS = None
LAST_RESULTS = None


def _build(SC):
    import bass_rust
    import concourse.mybir as mybir
    import concourse.tile as tile
    from concourse import bacc
    from concourse.masks import make_identity

    _add_dep = bass_rust.add_dep_helper

    F32 = mybir.dt.float32
    FP16 = mybir.dt.float16
    ALU = mybir.AluOpType
    AXL = mybir.AxisListType
    ACT_EXP = mybir.ActivationFunctionType.Exp

    nSC = SC // P  # compacted source tiles
    # mm2 moving-operand chunks over the SC free dim (PSUM banks are 512
    # f32, so chunk boundaries stay bank-aligned at 512)
    sc_chunks = []
    off = 0
    while off < SC:
        w = min(512, SC - off)
        sc_chunks.append((off, off + w))
        off += w

    nc = bacc.Bacc("TRN2", target_bir_lowering=False, debug=False)

    whidA_d = nc.dram_tensor("whidA", (nD, P, 3, 512), FP16,
                             kind="ExternalInput")
    wB_d = nc.dram_tensor("wB", (P, nD, 512), FP16, kind="ExternalInput")
    encT_d = nc.dram_tensor("encT", (BPC, P, nE, SC), FP16,
                            kind="ExternalInput")
    val_d = nc.dram_tensor("val", (BPC, P, nSC, TRGD), FP16,
                           kind="ExternalInput")
    mask_d = nc.dram_tensor("mask", (BPC, 1, SC), FP16, kind="ExternalInput")
    aeaw_d = nc.dram_tensor("aeaw", (BPC, TRG, 2 * SC), FP16,
                            kind="ExternalOutput")
    ctx_d = nc.dram_tensor("ctx", (BPC, TRG, TRGD), FP16,
                           kind="ExternalOutput")

    with tile.TileContext(nc) as tc:
        with (
            tc.tile_pool(name="const", bufs=1) as const,
            tc.tile_pool(name="big", bufs=1) as big,
            tc.tile_pool(name="sm", bufs=2) as sm,
            tc.tile_pool(name="ps", bufs=4, space="PSUM") as psp,
        ):
            # junk-warmup scratch: a bare memset is ready ~1.3us before the
            # identity (memset+affine_select+copy), so the HAM warmup can
            # start that much sooner.  On DVE so it can be hoisted before
            # the entry barrier (see the block surgery below).
            scr = const.tile([P, P], FP16)
            scr_set = nc.vector.memset(scr[:], 0.0)
            ident = const.tile([P, P], F32)
            make_identity(nc, ident[:])
            identh = const.tile([P, P], FP16)
            nc.vector.tensor_copy(identh[:], ident[:])

            # PE program order is pinned with an explicit linear chain so the
            # scheduler can never interleave accumulation groups or delay a
            # group's stop.
            pe_prev = [None]

            def chain(mm):
                if pe_prev[0] is not None:
                    _add_dep(mm.ins, pe_prev[0].ins, sync=False,
                             reason="pe order")
                pe_prev[0] = mm
                return mm

            # ---- loads (issue order == consumption order) ----
            # whidA[dt] carries w[dt] cols 0:512 (all pass-1 needs) plus the
            # full hidT[dt]; pass-2's w halves ride later as ONE fused 1MB
            # DMA.  This thins mm1's supply-critical stream by 25% and cuts
            # Sync's serial issue count (measured ~0.68us per dma_start,
            # ~0.8us DMA-completion-to-semaphore lag).
            whidA_sb = [big.tile([P, 3, 512], FP16, tag=f"whidA{i}",
                                 name=f"whidA_sb{i}") for i in range(nD)]
            wB_sb = big.tile([P, nD, 512], FP16, tag="wB", name="wB_sb")
            # whidA[0] split: (w0-half + hid0-h0) first so round0-h0 starts
            # on 0.25MB; hid0-h1 follows whidA[1] in supply order to match
            # the reordered round-0/round-1 consumption below
            # All loads on Sync's HW-DGE in consumption order.  The Scalar
            # queue (Q10) measured a ~4us first-use cold-start lag, so it
            # must NOT carry supply-critical loads; Sync's queue is warmed
            # by the framework preamble.  The first three issues are
            # hoisted before the entry barrier's release-wait (Sync's
            # barrier-arrival drain completes ~6.2us, ~0.4us before the
            # release fires).
            early_dmas = [
                nc.sync.dma_start(out=whidA_sb[0][:, 0:2, :],
                                  in_=whidA_d[0][:, 0:2, :]),
                nc.sync.dma_start(out=whidA_sb[1][:], in_=whidA_d[1]),
                nc.sync.dma_start(out=whidA_sb[0][:, 2, :],
                                  in_=whidA_d[0][:, 2, :]),
            ]
            for i in range(2, nD):
                nc.sync.dma_start(out=whidA_sb[i][:], in_=whidA_d[i])
            nc.sync.dma_start(out=wB_sb[:], in_=wB_d[:])
            maskbs = []
            for b in range(BPC):
                maskb_hf = sm.tile([P, SC], FP16, tag="maskb_hf",
                                   name=f"maskb_hf{b}")
                nc.sync.dma_start(out=maskb_hf[:],
                                  in_=mask_d[b].to_broadcast((P, SC)))
                maskbs.append(maskb_hf)
            # encT/val are host-compacted to valid s-columns (padding zeros)
            # and marshaled partition-major, one DMA each per batch
            encT_sb = []
            val_sb = []
            for b in range(BPC):
                e_t = big.tile([P, nE, SC], FP16, tag="encT", bufs=2,
                               name=f"encT_sb{b}")
                nc.sync.dma_start(out=e_t[:], in_=encT_d[b])
                v_t = big.tile([P, nSC, TRGD], FP16, tag="val", bufs=2,
                               name=f"val_sb{b}")
                nc.sync.dma_start(out=v_t[:], in_=val_d[b])
                encT_sb.append(e_t)
                val_sb.append(v_t)

            # ---- mm1: HpT[e, t01] = sum_d W[d,e] * hidT01[d, t01] ----
            # two half-passes of 4 et each (4 psum bufs per pass).  dt-outer
            # for DMA pair-wise consumption, but the last two dt rounds go
            # et-wise with the drain right after each stop so ring slots free
            # up staggered instead of all at the end.
            HpT = big.tile([P, nE, TRG2], FP16, tag="HpT", name="HpT")
            drain_eng = [0]

            def drain(dst, src):
                if drain_eng[0] % 2 == 0:
                    nc.vector.tensor_copy(dst, src)
                else:
                    nc.scalar.copy(dst, src)
                drain_eng[0] += 1

            def w_ap(dt, et):
                if et < 4:
                    return whidA_sb[dt][:, 0, et * P:(et + 1) * P]
                return wB_sb[:, dt, (et - 4) * P:(et - 3) * P]

            def mm1_mm(pp, dt, et):
                for h in range(2):
                    hs = slice(h * 512, (h + 1) * 512)
                    chain(nc.tensor.matmul(
                        pp[:, hs], w_ap(dt, et),
                        whidA_sb[dt][:, 1 + h, :],
                        start=(dt == 0), stop=(dt == nD - 1)))

            def emit_mm1_pass(ets, warm=False):
                pps = [psp.tile([P, TRG2], F32, tag="ps", name=f"mm1ps{et}")
                       for et in ets]
                if warm:
                    # junk transposes of the identity: keep the PE busy
                    # during the DMA/preamble dead time so the HAM clock
                    # gate is warm (2.4GHz) when real work arrives, sized
                    # to end right as whid[0]'s first half lands (~10.2us).
                    # An idle gap here is doubly bad: the wait itself plus
                    # ~11 cold 427ns matmuls after it (v5 measured).  The
                    # garbage psum is overwritten by mm1's start=True.
                    junk_view = pps[0][:].bitcast(FP16)
                    for _ in range(28):
                        chain(nc.tensor.matmul(
                            junk_view[:, 0:P], scr[:], scr[:],
                            is_transpose=True, skip_group_check=True))
                    # supply-ordered head: round0-h0 (whidA[0]'s first
                    # 2/3rds), then ALL of round 1 (whidA[1], issued 2nd),
                    # then round0-h1 (hid0-h1, issued 3rd).  f32 psum
                    # accumulation is order-independent; bank B's group is
                    # opened (start=True) by dt1-h1 since it now runs first.
                    h0, h1 = slice(0, 512), slice(512, 1024)
                    for i, et in enumerate(ets):
                        chain(nc.tensor.matmul(
                            pps[i][:, h0], w_ap(0, et),
                            whidA_sb[0][:, 1, :], start=True, stop=False))
                    for i, et in enumerate(ets):
                        chain(nc.tensor.matmul(
                            pps[i][:, h0], w_ap(1, et),
                            whidA_sb[1][:, 1, :], start=False, stop=False))
                        chain(nc.tensor.matmul(
                            pps[i][:, h1], w_ap(1, et),
                            whidA_sb[1][:, 2, :], start=True, stop=False))
                    for i, et in enumerate(ets):
                        chain(nc.tensor.matmul(
                            pps[i][:, h1], w_ap(0, et),
                            whidA_sb[0][:, 2, :], start=False, stop=False))
                    dt_start = 2
                else:
                    dt_start = 0
                for dt in range(dt_start, nD - 2):
                    for i, et in enumerate(ets):
                        mm1_mm(pps[i], dt, et)
                for i, et in enumerate(ets):
                    for dt in (nD - 2, nD - 1):
                        mm1_mm(pps[i], dt, et)
                    drain(HpT[:, et, :], pps[i][:])

            emit_mm1_pass(range(0, nE // 2), warm=True)
            emit_mm1_pass(range(nE // 2, nE))

            # ---- mm2 + masked softmax over 8 supertiles (b, tt) ----
            tiles = [(b, tt) for b in range(BPC) for tt in range(nT)]
            pks = []
            attnTs = {}

            def emit_mm2(k):
                b, tt = tiles[k]
                ts = slice(b * TRG + tt * P, b * TRG + (tt + 1) * P)
                en_ps = psp.tile([P, SC], F32, tag="ps", name=f"en{b}{tt}")
                for et in range(nE):
                    for c0, c1 in sc_chunks:
                        chain(nc.tensor.matmul(en_ps[:, c0:c1],
                                               HpT[:, et, ts],
                                               encT_sb[b][:, et, c0:c1],
                                               start=(et == 0),
                                               stop=(et == nE - 1)))
                return en_ps

            def emit_softmax(k, en_ps):
                b, tt = tiles[k]
                maskb_hf = maskbs[b]
                # packed [ae | attn] tile: one output DMA per supertile
                # ring of 8 (one per supertile): with 7, pk(7)'s slot waits
                # on tr(0)'s reads of pk(0), which land just before
                # softmax(7) needs the slot — zero margin at the B->C
                # boundary
                pk = sm.tile([P, 2 * SC], FP16, tag="aeaw", bufs=8,
                             name=f"aeaw{b}{tt}")
                nc.scalar.copy(pk[:, :SC], en_ps[:])
                negm = sm.tile([P, 1], F32, tag="negm")
                nc.vector.tensor_reduce(negm[:], en_ps[:], axis=AXL.X,
                                        op=ALU.max, negate=True)
                ex = sm.tile([P, SC], FP16, tag="ex", bufs=3)
                nc.scalar.activation(ex[:], en_ps[:], ACT_EXP, bias=negm[:],
                                     scale=1.0)
                rowsum = sm.tile([P, 1], F32, tag="rowsum")
                nc.vector.scalar_tensor_tensor(ex[:], ex[:], 1.0,
                                               maskb_hf[:],
                                               op0=ALU.mult, op1=ALU.mult,
                                               accum_out=rowsum[:])
                z = sm.tile([P, 1], F32, tag="z")
                nc.vector.tensor_scalar_add(z[:], rowsum[:], 1e-6)
                rz = sm.tile([P, 1], F32, tag="rz")
                nc.vector.reciprocal(rz[:], z[:])
                nc.vector.tensor_scalar_mul(pk[:, SC:], ex[:], rz[:])
                nc.sync.dma_start(out=aeaw_d[b, tt * P:(tt + 1) * P, :],
                                  in_=pk[:])
                pks.append(pk)

            def emit_tr(k):
                # PE transposes: the DMA-xbar alternative measures ~5-6us
                # per [128,1024] tile on hardware and serializes — PE does
                # all of them in well under 1us
                attn = pks[k][:, SC:]
                trp = psp.tile([P, SC], F32, tag="ps", name=f"tr{k}")
                trh = trp[:].bitcast(FP16)
                for st in range(nSC):
                    chain(nc.tensor.transpose(trh[:, st * P:(st + 1) * P],
                                              attn[:, st * P:(st + 1) * P],
                                              identh[:]))
                # ring of 3: with 2, each attnT Scalar copy waits for
                # mm3(k-2)'s last matmul to release its slot, leaving zero
                # margin at the C-phase tail; 3 buys one extra mm3 round
                attnT = sm.tile([P, nSC, P], FP16, tag="attnT", bufs=3,
                                name=f"attnT{k}")
                nc.scalar.copy(attnT[:], trh[:, :SC])
                attnTs[k] = attnT

            for k in range(len(tiles)):
                if k == len(tiles) - 1:
                    # tr(T0) goes BEFORE mm2(T7) on the PE: its attnT copy
                    # (which also sits ahead of T7's softmax in the Scalar
                    # FIFO) then completes under mm2(T7)'s ~3us, so mm3(T0)
                    # starts with zero gap at the B->C boundary
                    emit_tr(0)
                en_ps = emit_mm2(k)
                emit_softmax(k, en_ps)

            def emit_mm3(k):
                b, tt = tiles[k]
                attnT = attnTs.pop(k)
                last = (k == len(tiles) - 1)
                ctx_sb = sm.tile([P, TRGD], FP16, tag="ctx_sb", bufs=3)
                rows = slice(tt * P, (tt + 1) * P)
                if last:
                    # shrinking chunks in 4 SEPARATE single-bank psum ring
                    # tiles (chunks sharing a tile serialize: the tile
                    # framework's write-after-read tracking made chunk c+1's
                    # matmuls wait for chunk c's drain — v5 measured 1.9us of
                    # stalls).  Each chunk drains and ships while the next
                    # multiplies in a different bank/tile, so the end-of-
                    # kernel tail is one 128-wide DVE copy + tiny DMA.
                    edges = [0, 384, 640, 896, 1024]
                    for c in range(4):
                        tp = psp.tile([P, 512], F32, tag="ps",
                                      name=f"ctx7_{c}")
                        wdt = edges[c + 1] - edges[c]
                        ps_sl = slice(0, wdt)
                        out_sl = slice(edges[c], edges[c + 1])
                        for st in range(nSC):
                            chain(nc.tensor.matmul(tp[:, ps_sl],
                                                   attnT[:, st, :],
                                                   val_sb[b][:, st, out_sl],
                                                   start=(st == 0),
                                                   stop=(st == nSC - 1)))
                        nc.vector.tensor_copy(ctx_sb[:, out_sl],
                                              tp[:, ps_sl])
                        if c < 3:
                            nc.sync.dma_start(out=ctx_d[b, rows, out_sl],
                                              in_=ctx_sb[:, out_sl])
                        else:
                            nc.scalar.dma_start(out=ctx_d[b, rows, out_sl],
                                                in_=ctx_sb[:, out_sl])
                    return
                ctx_ps = psp.tile([P, TRGD], F32, tag="ps", name=f"ctx{k}")
                for st in range(nSC):
                    for h in range(2):
                        hs = slice(h * 512, (h + 1) * 512)
                        chain(nc.tensor.matmul(ctx_ps[:, hs],
                                               attnT[:, st, :],
                                               val_sb[b][:, st, hs],
                                               start=(st == 0),
                                               stop=(st == nSC - 1)))
                # DVE, not Scalar: the Scalar FIFO still holds late
                # attn-transpose issues during early mm3 tiles
                nc.vector.tensor_copy(ctx_sb[:], ctx_ps[:])
                nc.sync.dma_start(out=ctx_d[b, rows, :], in_=ctx_sb[:])

            # transposes run two tiles ahead of mm3: each attnT Scalar copy
            # then has two full mm3 rounds of cover before its consumer
            emit_tr(1)
            for k in range(len(tiles)):
                if k + 2 < len(tiles):
                    emit_tr(k + 2)
                emit_mm3(k)

    # ---- entry-barrier hoist ----
    # The Bacc-init all-engine barrier gates the first DMA issue at ~7.2us
    # into the graded window while Sync sits idle from ~5.9us.  Hoist the
    # three supply-critical DMA issues to between Sync's barrier-arrival
    # DRAIN and its barrier-release wait (after the drain, so the barrier
    # never waits on the transfers), and the junk-scratch memset likewise on
    # DVE.  Semaphores are zeroed by the NEFF preamble before any module
    # instruction, and each hoisted instruction carries its sync_info with
    # it; instructions with semaphore waits are left in place.
    il0 = nc.main_func.blocks[0].instructions

    def hoist_before(inst, anchor_name_prefix):
        # Move a body-block instruction into the init block, right before
        # its engine's entry-barrier release-wait (and hence after that
        # engine's barrier-arrival DRAIN — a DRAIN waits on the engine's
        # outstanding DMAs, so issuing before it would stall the barrier).
        dbg = os.environ.get("KERNEL_HOIST_DEBUG")
        si = inst.sync_info
        if si is not None and len(si.on_wait) > 0:
            if dbg:
                print(f"HOIST-SKIP waits {inst.name}: {si.on_wait}")
            return
        src = None
        for b in nc.main_func.blocks:
            if inst in b.instructions:
                src = b
                break
        if src is None:
            if dbg:
                print(f"HOIST-SKIP notfound {inst.name}")
            return
        for k, x in enumerate(il0):
            if type(x).__name__ == "InstEventSemaphore" and str(
                    x.name).startswith(anchor_name_prefix):
                src.instructions.remove(inst)
                il0.insert(k, inst)
                if dbg:
                    print(f"HOIST-OK {inst.name} -> before {x.name} @{k}")
                return
        if dbg:
            print(f"HOIST-SKIP noanchor {inst.name}")

    hoist_before(scr_set.ins, "barrier_DVE_")
    for d in early_dmas:
        hoist_before(d.ins, "barrier_SP_")

    nc.compile()
    return nc


def kernel(hidden, encoder_outputs, encoder_value, encoder_mask, W):
    global LAST_EXEC_NS, LAST_RESULTS
    from concourse.bass_utils import run_bass_kernel_spmd

    hidden = np.ascontiguousarray(hidden, dtype=np.float32)
    encoder_outputs = np.ascontiguousarray(encoder_outputs, dtype=np.float32)
    encoder_value = np.ascontiguousarray(encoder_value, dtype=np.float32)
    encoder_mask = np.ascontiguousarray(encoder_mask, dtype=np.float32)
    W = np.ascontiguousarray(W, dtype=np.float32)

    # s-compaction: gather valid source positions per batch; SC is the max
    # valid count rounded up to 128 (uniform across cores — SPMD shares one
    # compiled program)
    idx = [np.nonzero(encoder_mask[b] > 0.5)[0] for b in range(B)]
    n_max = max(len(ix) for ix in idx)
    SC = min(SRC, -(-max(n_max, 1) // P) * P)
    nSC = SC // P

    if ("nc", SC) not in _cache:
        _cache[("nc", SC)] = _build(SC)
    nc = _cache[("nc", SC)]

    w_tiles = W.astype(np.float16).reshape(nD, P, ENCD)
    # (P, nD, 512) to match the SBUF tile's linear order — DMA does not
    # transpose
    wB = np.ascontiguousarray(w_tiles[:, :, 512:].transpose(1, 0, 2))
    in_maps = []
    for c in range(NCORES):
        sl = slice(c * BPC, (c + 1) * BPC)
        hid2 = hidden[sl]  # (2, TRG, TRGD)
        hidT01 = np.concatenate([hid2[0].T, hid2[1].T], axis=1)
        hid_tiles = hidT01.astype(np.float16).reshape(nD, P, TRG2)
        whidA = np.empty((nD, P, 3, 512), dtype=np.float16)
        whidA[:, :, 0, :] = w_tiles[:, :, 0:512]
        whidA[:, :, 1, :] = hid_tiles[:, :, 0:512]
        whidA[:, :, 2, :] = hid_tiles[:, :, 512:]
        encT_pm = np.zeros((BPC, P, nE, SC), dtype=np.float16)
        val_pm = np.zeros((BPC, P, nSC, TRGD), dtype=np.float16)
        maskc = np.zeros((BPC, 1, SC), dtype=np.float16)
        for j in range(BPC):
            b = c * BPC + j
            ix = idx[b]
            n_b = len(ix)
            # encT compact: [ENCD, n_b] -> pad to SC -> partition-major
            encTc = encoder_outputs[b].T[:, ix].astype(np.float16)
            encT_pm[j, :, :, :n_b] = encTc.reshape(nE, P, n_b).transpose(
                1, 0, 2)
            # val compact: [n_b, TRGD] -> pad to SC rows -> partition-major
            valc = np.zeros((SC, TRGD), dtype=np.float16)
            valc[:n_b] = encoder_value[b][ix]
            val_pm[j] = valc.reshape(nSC, P, TRGD).transpose(1, 0, 2)
            maskc[j, 0, :n_b] = 1.0
        in_maps.append({
            "whidA": whidA,
            "wB": wB,
            "encT": encT_pm,
            "val": val_pm,
            "mask": maskc,
        })

    trace = bool(int(os.environ.get("KERNEL_TRACE", "0")))
    res = run_bass_kernel_spmd(nc, in_maps, core_ids=list(range(NCORES)),
                               trace=trace)
    LAST_EXEC_NS = res.exec_time_ns
    LAST_RESULTS = res

    context = np.concatenate([res.results[c]["ctx"] for c in range(NCORES)],
                             axis=0).astype(np.float32)
    # scatter compacted ae/aw back to full source width; masked positions
    # are exact zeros in the reference (energies*mask and e*mask)
    attn_energies = np.zeros((B, TRG, SRC), dtype=np.float32)
    attn_weights = np.zeros((B, TRG, SRC), dtype=np.float32)
    for c in range(NCORES):
        aeaw = res.results[c]["aeaw"]
        for j in range(BPC):
            b = c * BPC + j
            ix = idx[b]
            n_b = len(ix)
            attn_energies[b][:, ix] = aeaw[j][:, :n_b].astype(np.float32)
            attn_weights[b][:, ix] = aeaw[j][:, SC:SC + n_b].astype(
                np.float32)
    return context, attn_weights, attn_energies
